# revision 18
# baseline (speedup 1.0000x reference)
"""Multi-head attention (B=4, S=2048, HID=1024, H=16, D=64) on 8 trn2 cores.

Sharding: batch x head-group (4 x 2). Core (2b+g) owns batch b and heads
8g..8g+7 over the FULL sequence: Q/K/V projections for its 8 heads,
attention, and a partial o-projection over its 512 value features. The host
sums the two partial o outputs per batch (the "all-reduce after o_proj"
done host-side) -- no duplicated projection work, no collectives.

Per-core dataflow (all matmuls full 128-partition moving operands, bf16,
fp32 PSUM accumulate -- avoids the half-bandwidth 64-partition moving path
and PE tiling-mode-switch drains):
  - K.T per pair packed [128=2x64 feat, token] bf16
  - Q.T per head zero-padded to [128, token] bf16 (other head's rows = 0),
    so logits contract over 128 partitions with the packed K stationary
  - V' in [token, (kt, head, 65)] bf16 with a ones column per head
    (softmax denominator falls out of the AV matmul as row 64)
  - logits L.T[k, q] in PSUM [128, 1024] (2 k-tiles); exp on ScalarE
  - AV accumulates vals'[65, 512] over 16 k-tiles; row 64 = denominator
  - denominator rows DMA'd from PSUM into a partition-major [8, 512] tile;
    ONE reciprocal per pair (free-size bound: 8x cheaper than reciprocal of
    broadcast tiles); bounced via DRAM back to a flat row, PE-broadcast,
    DVE multiply into vn
  - o_proj tail: vn (bf16) @ w_o.T shard (bf16) over 4 feature chunks

Schedule (this session's changes):
  - startup: ones constant loads first and feeds ~5us of PE clock-warmup
    junk; x loads in token-block strips so the pair-0 projection streams
    behind the DMA instead of waiting for the whole 4MB
  - pair 3 is pipelined per query-block: both heads' attention for qb,
    o-proj partial chunks 0-2 for qb's tokens, ACT-based reciprocal chains
    (1/x = exp(-ln x)) one qb late, then normalize + chunk-3 + add + store
    for qb-2; the output DMA streams during pair-3 attention
"""
import contextlib
import sys
sys.path.insert(0, "/opt/trn_rl_repo")
import numpy as np

import concourse.bass as bass
import concourse.mybir as mybir
import concourse.tile as tile
from concourse import bacc
from concourse.bass_utils import run_bass_kernel_spmd

F32 = mybir.dt.float32
F32R = mybir.dt.float32r
BF16 = mybir.dt.bfloat16
EXP = mybir.ActivationFunctionType.Exp

B, S, HID, H, D = 4, 2048, 1024, 16, 64
G = 2                  # head groups (cores per batch)
HG = H // G            # 8 heads per core
NPAIR = HG // 2        # 4 head pairs per core
HT = HID // 128        # 8 hid contraction tiles
TB = S // 512          # 4 proj token blocks
KT = S // 128          # 16 key-token tiles
QB = S // 512          # 4 query blocks of 512
N_CORES = 8


def build_nc(n_iter: int = 1):
    nc = bacc.Bacc(None, target_bir_lowering=False)

    xt = nc.dram_tensor("xt", [HID, S], BF16, kind="ExternalInput")
    wq = nc.dram_tensor("wq", [NPAIR * HID, 128], BF16, kind="ExternalInput")
    wk = nc.dram_tensor("wk", [NPAIR * HID, 128], BF16, kind="ExternalInput")
    wv = nc.dram_tensor("wv", [HID, HG * D], BF16, kind="ExternalInput")
    wo = nc.dram_tensor("wo", [HG * D, HID], BF16, kind="ExternalInput")
    cone8 = nc.dram_tensor("cone8", [128, 512], BF16, kind="ExternalInput")
    cone = nc.dram_tensor("cone", [1, 64], BF16, kind="ExternalInput")
    o = nc.dram_tensor("o", [S, HID], F32, kind="ExternalOutput")

    with tile.TileContext(nc) as tc:
        def body():
            with contextlib.ExitStack() as _st:
                constp = _st.enter_context(tc.tile_pool(name="const", bufs=1))
                xtp = _st.enter_context(tc.tile_pool(name="xtp", bufs=1))
                vtp = _st.enter_context(tc.tile_pool(name="vtp", bufs=1))
                vnp = _st.enter_context(tc.tile_pool(name="vnp", bufs=1))
                wop = _st.enter_context(tc.tile_pool(name="wop", bufs=1))
                ktqp = _st.enter_context(tc.tile_pool(name="ktqp", bufs=1))
                opsb = _st.enter_context(tc.tile_pool(name="opsb", bufs=32))
                # ones constant doubles as PE clock-warmup fodder: it is the
                # FIRST dma (128KB, lands ~1us) so the junk matmuls below can
                # warm the HAM clock gate while the 5.8MB of real inputs load
                ones8_sb = constp.tile([128, 512], BF16)
                nc.sync.dma_start(ones8_sb[:], cone8[:])
                ones_sb = constp.tile([1, 64], BF16)
                nc.sync.dma_start(ones_sb[:], cone[:])

                # x resident in SBUF, TOKEN-BLOCK-major: strip tb holds hid
                # tiles 0-7 for tokens [512*tb, 512*(tb+1)) contiguously, so
                # each strip's DMA write range is exact (no false deps) and
                # the pair-0 projection streams behind the x load strip by
                # strip instead of waiting for the whole 4MB.
                xt_all = xtp.tile([128, HT * S], BF16, name="xtall")

                def xt_at(ht, tok, width):
                    tb, off = divmod(tok, 512)
                    assert off + width <= 512
                    base = HT * 512 * tb + 512 * ht + off
                    return xt_all[:, base:base + width]

                wo_all = wop.tile([128, NPAIR * HID], BF16, name="woall")
                wo_sb = [wo_all[:, HID * c:HID * (c + 1)] for c in range(NPAIR)]

                # V' [token, (kt, head, 65)] bf16, resident in SBUF
                vt = vtp.tile([128, KT * HG * 65], BF16)
                vt4 = vt.rearrange("p (t h c) -> p t h c", h=HG, c=65)
                # normalized values [feat(128=2 heads), pair-chunk, token]
                vn_all = vnp.tile([128, NPAIR * S], BF16)

                # persistent K/Q tiles, double-buffered across pairs.
                # qt_h zero-halves are memset once and never overwritten.
                kt_t = [ktqp.tile([128, S], BF16, name=f"kt{i}") for i in range(2)]
                qt_t = [[ktqp.tile([128, S], BF16, name=f"qt{i}{h2}") for h2 in range(2)]
                        for i in range(2)]
                for i in range(2):
                    nc.any.memset(qt_t[i][0][64:128, :], 0.0)
                    nc.any.memset(qt_t[i][1][0:64, :], 0.0)

                with contextlib.ExitStack() as _st2:
                    wvp = _st2.enter_context(tc.tile_pool(name="wvp", bufs=1))
                    wkp = _st2.enter_context(tc.tile_pool(name="wkp", bufs=2))
                    wqp = _st2.enter_context(tc.tile_pool(name="wqp", bufs=2))
                    ptp = _st2.enter_context(tc.tile_pool(name="ptp", bufs=3))
                    vap = _st2.enter_context(tc.tile_pool(name="vap", bufs=18))
                    nrm = _st2.enter_context(tc.tile_pool(name="nrm", bufs=2))
                    nrm3 = _st2.enter_context(tc.tile_pool(name="nrm3", bufs=4))
                    obp = _st2.enter_context(tc.tile_pool(name="obp", bufs=2))
                    dramp = _st2.enter_context(tc.tile_pool(name="dramp", bufs=3, space="DRAM"))
                    psP = _st2.enter_context(tc.tile_pool(name="psP", bufs=2, space="PSUM"))
                    psL = _st2.enter_context(tc.tile_pool(name="psL", bufs=2, space="PSUM"))
                    psV = _st2.enter_context(tc.tile_pool(name="psV", bufs=2, space="PSUM"))
                    def proj_dma(j, w_dram, pool):
                        w_p = pool.tile([128, HID], BF16, tag="wp")
                        nc.sync.dma_start(
                            w_p.rearrange("p (t c) -> p t c", c=128),
                            w_dram[HID * j:HID * (j + 1), :].rearrange("(t p) c -> p t c", p=128),
                        )
                        return w_p

                    # DMA queue order = emission order: pair-0 weights, then x
                    # in TOKEN-BLOCK strips so the pair-0 projection (which
                    # consumes x token-block by token-block) can start after
                    # ~1.6MB instead of waiting for the whole 4MB x load.
                    wk0 = proj_dma(0, wk, wkp)
                    wq0 = proj_dma(0, wq, wqp)
                    xsrc = xt.rearrange("(t p) s -> p t s", p=128)
                    for tb in range(TB):
                        nc.sync.dma_start(
                            xt_all[:, HT * 512 * tb:HT * 512 * (tb + 1)]
                            .rearrange("p (t c) -> p t c", c=512),
                            xsrc[:, :, 512 * tb:512 * (tb + 1)],
                        )
                    wv_all = wvp.tile([128, HT * HG * D], BF16, name="wvall")
                    wv_sb = [wv_all[:, HG * D * t:HG * D * (t + 1)] for t in range(HT)]
                    nc.sync.dma_start(
                        wv_all.rearrange("p (t c) -> p t c", t=HT),
                        wv.rearrange("(t p) c -> p t c", p=128),
                    )
                    nc.sync.dma_start(
                        wo_all.rearrange("p (c h) -> p c h", c=NPAIR),
                        wo.rearrange("(c p) h -> p c h", p=128),
                    )

                    # prewarm the activation tables while input DMAs are in
                    # flight -- Ln first pins the natural_log_exp_and_others
                    # set, which also serves every Exp (no mid-kernel reload)
                    warm = nrm.tile([1, 8], BF16, tag="warm")
                    nc.scalar.activation(warm[:], ones8_sb[0:1, 0:8],
                                         mybir.ActivationFunctionType.Ln)
                    nc.scalar.activation(warm[:], ones8_sb[0:1, 0:8], EXP, scale=0.0)
                    # warm the PE HAM clock gate on the ones constant (lands
                    # ~1us, far ahead of x): ~5us of junk matmuls so the first
                    # projection runs at 2.4GHz, fully overlapped with the
                    # x/wv/wo input DMAs
                    for _ in range(10):
                        jps = psP.tile([128, 512], F32, tag="pp")
                        nc.tensor.matmul(
                            jps[:], ones8_sb[:, 0:128], ones8_sb[:],
                            start=True, stop=True,
                        )

                    def v_proj(tokt):
                        vps = psP.tile([128, 512], F32, tag="pp")
                        for ht in range(HT):
                            nc.tensor.matmul(
                                vps[:],
                                xt_at(ht, 128 * tokt, 128),
                                wv_sb[ht][:],
                                start=(ht == 0), stop=(ht == HT - 1),
                            )
                        nc.vector.tensor_copy(
                            vt4[:, tokt, :, 0:64],
                            vps.rearrange("p (h c) -> p h c", c=64),
                        )
                        nc.vector.tensor_copy(vt4[:, tokt, :, 64], ones8_sb[:, 0:8])

                    # ---- pair pipeline: K/Q proj + attention + normalize ----
                    va_tiles = {}

                    def proj_pair(w_p, evac):
                        for tb in range(TB):
                            pps = psP.tile([128, 512], F32, tag="pp")
                            for ht in range(HT):
                                nc.tensor.matmul(
                                    pps[:],
                                    w_p[:, 128 * ht:128 * (ht + 1)],
                                    xt_at(ht, 512 * tb, 512),
                                    start=(ht == 0), stop=(ht == HT - 1),
                                )
                            evac(tb, pps)

                    def attn_block(j, h2, qb, kt_sb, qt_sb, den_out, v_inter=False):
                        h = 2 * j + h2
                        first_mm = None
                        vals = psV.tile([65, 512], F32, tag="vv")
                        for ktp2 in range(KT // 2):
                            lg = psL.tile([128, 1024], F32, tag="lg")
                            for u in range(2):
                                kt = 2 * ktp2 + u
                                mm = nc.tensor.matmul(
                                    lg[:, 512 * u:512 * (u + 1)],
                                    kt_sb[:, 128 * kt:128 * (kt + 1)],
                                    qt_sb[:, 512 * qb:512 * (qb + 1)],
                                    start=True, stop=True,
                                )
                                if first_mm is None:
                                    first_mm = mm
                            pt = ptp.tile([128, 1024], BF16, tag="pt")
                            nc.scalar.activation(pt[:], lg[:], EXP, scale=0.125)
                            if v_inter:
                                # first attention block: V' projection for these
                                # two k-tiles lands just ahead of their AV use
                                v_proj(2 * ktp2)
                                v_proj(2 * ktp2 + 1)
                            for u in range(2):
                                kt = 2 * ktp2 + u
                                nc.tensor.matmul(
                                    vals[:],
                                    vt4[:, kt, h, :],
                                    pt[:, 512 * u:512 * (u + 1)],
                                    start=(ktp2 == 0 and u == 0),
                                    stop=(ktp2 == KT // 2 - 1 and u == 1),
                                )
                        nc.vector.tensor_copy(den_out, vals[64:65, :])
                        va = vap.tile([64, 512], BF16, tag="va")
                        nc.vector.tensor_copy(va[:], vals[0:64, :])
                        va_tiles[8 * j + 4 * h2 + qb] = va
                        return first_mm

                    rec_drams = {}
                    rec_flats = {}

                    def chain(j, h2, den_flat, use_act=False):
                        # reciprocal of this half-pair's 4 denominator rows.
                        # engine ops cannot address partitions 1..31, so either
                        # bounce through DRAM to partition-major for the DVE
                        # reciprocal, or (for the last chain, when the scalar
                        # engine has gone idle) reciprocal the flat row on ACT.
                        if use_act:
                            # 1/x = exp(-ln x): two ACT ops on the flat row --
                            # the scalar engine is idle by the last chain and
                            # this skips two DMA bounce hops on the o-proj
                            # critical path (both fns live in the preloaded
                            # natural_log_exp table set)
                            lnt = nrm.tile([1, 4 * 512], F32, tag="lnt")
                            nc.scalar.activation(lnt[:], den_flat[:],
                                                 mybir.ActivationFunctionType.Ln)
                            rec_flat = nrm.tile([1, 4 * 512], BF16, tag="rflat")
                            nc.scalar.activation(rec_flat[:], lnt[:], EXP, scale=-1.0)
                            rec_flats[(j, h2)] = rec_flat
                        else:
                            rec_dram = dramp.tile([4, 512], BF16, tag="rdram")
                            den_dram = dramp.tile([4, 512], BF16, tag="ddram")
                            nc.sync.dma_start(
                                den_dram.rearrange("r c -> (r c)")[None, :], den_flat[0:1, :]
                            )
                            den_sq = nrm.tile([4, 512], BF16, tag="dsq")
                            nc.sync.dma_start(den_sq[:], den_dram[:])
                            rec_sq = nrm.tile([4, 512], BF16, tag="rsq")
                            with nc.allow_low_precision(reason="denominator reciprocal in bf16"):
                                nc.vector.reciprocal(rec_sq[:], den_sq[:])
                            nc.sync.dma_start(rec_dram[:], rec_sq[:])
                            rec_drams[(j, h2)] = rec_dram

                    def normalize(j, h2):
                        # runs a half-pair or more late: the reciprocal chain
                        # has had a full attention half to complete, so nothing
                        # here blocks the in-order engine streams. The last
                        # chain instead broadcasts via a PE matmul straight
                        # from the flat reciprocal row (PE is idle by then and
                        # this skips two DMA hops on the o-proj critical path).
                        rec_flat = rec_flats.pop((j, h2), None)
                        rec_dram = None if rec_flat is not None else rec_drams.pop((j, h2))
                        for qb in range(QB):
                            if rec_flat is not None:
                                bcp = psL.tile([64, 512], F32, tag="lg", name="bcp")
                                nc.tensor.matmul(
                                    bcp[:], ones_sb[:],
                                    rec_flat[0:1, 512 * qb:512 * (qb + 1)],
                                    start=True, stop=True,
                                )
                                bop = bcp
                            else:
                                bcs = nrm.tile([64, 512], BF16, tag="bcs")
                                nc.sync.dma_start(
                                    bcs[:], rec_dram[qb:qb + 1, :].broadcast_to([64, 512])
                                )
                                bop = bcs
                            nc.vector.tensor_mul(
                                vn_all[64 * h2:64 * (h2 + 1),
                                       S * j + 512 * qb:S * j + 512 * (qb + 1)],
                                va_tiles.pop(8 * j + 4 * h2 + qb)[:],
                                bop[:],
                            )

                    def k_evac_f(kt_sb):
                        def k_evac(tb, pps):
                            nc.vector.tensor_copy(kt_sb[:, 512 * tb:512 * (tb + 1)], pps[:])
                        return k_evac

                    def q_evac_f(jj):
                        def q_evac(tb, pps):
                            nc.vector.tensor_copy(
                                qt_t[jj % 2][0][0:64, 512 * tb:512 * (tb + 1)], pps[0:64, :])
                            nc.vector.tensor_copy(
                                qt_t[jj % 2][1][64:128, 512 * tb:512 * (tb + 1)], pps[64:128, :])
                        return q_evac

                    proj_pair(wk0, k_evac_f(kt_t[0]))
                    proj_pair(wq0, q_evac_f(0))
                    for j in range(NPAIR - 1):
                        kt_sb = kt_t[j % 2]
                        for h2 in range(2):
                            den_flat = nrm.tile([1, 4 * 512], BF16, tag="dflat")
                            for qb in range(QB):
                                attn_block(j, h2, qb, kt_sb, qt_t[j % 2][h2],
                                           den_flat[0:1, 512 * qb:512 * (qb + 1)],
                                           v_inter=(j == 0 and h2 == 0 and qb == 0))
                            chain(j, h2, den_flat, use_act=False)
                        normalize(j, 0)
                        proj_pair(proj_dma(j + 1, wk, wkp), k_evac_f(kt_t[(j + 1) % 2]))
                        proj_pair(proj_dma(j + 1, wq, wqp), q_evac_f(j + 1))
                        normalize(j, 1)

                    # ---- pair 3, pipelined per query-block ----
                    # No projections remain to fill PE slack, so the o-proj
                    # partials (chunks 0-2, which need only pairs 0-2's vn)
                    # interleave per-qb here, and each qb's reciprocal chain /
                    # normalize / chunk-3 / add / store overlaps the NEXT qb's
                    # ACT-paced attention instead of serializing at the end.
                    j3 = NPAIR - 1
                    kt_sb3 = kt_t[j3 % 2]
                    vn3 = vn_all.rearrange("p (c s) -> p c s", c=NPAIR)
                    op_tiles = {}
                    rec3 = {}
                    den3 = {}

                    def partial_chunk(qb):
                        for tokb in range(4 * qb, 4 * qb + 4):
                            for ob in range(2):
                                opp = psP.tile([128, 512], F32, tag="pp")
                                for c in range(NPAIR - 1):
                                    nc.tensor.matmul(
                                        opp[:],
                                        vn3[:, c, 128 * tokb:128 * (tokb + 1)],
                                        wo_sb[c][:, 512 * ob:512 * (ob + 1)],
                                        start=(c == 0), stop=(c == NPAIR - 2),
                                    )
                                op_sb = opsb.tile([128, 512], BF16, tag="op")
                                nc.vector.tensor_copy(op_sb[:], opp[:])
                                op_tiles[(tokb, ob)] = op_sb

                    def chain3(h2, qb):
                        # 1/x = exp(-ln x) on ACT: emitted one qb late so the
                        # ACT queue never stalls waiting for this qb's AV to
                        # finish; keeps the tail chain off the DVE entirely
                        den = den3.pop((h2, qb))
                        lnt = nrm3.tile([1, 512], F32, tag="lnt")
                        nc.scalar.activation(lnt[:], den[0:1, :],
                                             mybir.ActivationFunctionType.Ln)
                        rec_flat = nrm3.tile([1, 512], BF16, tag="rflat")
                        nc.scalar.activation(rec_flat[:], lnt[:], EXP, scale=-1.0)
                        rec3[(h2, qb)] = rec_flat

                    def finish(qb):
                        # normalize both halves for this qb (PE row-broadcast
                        # of the reciprocal + DVE mul), then chunk-3 o-proj,
                        # add the banked partial, and store this token range
                        for h2 in range(2):
                            bcp = psL.tile([64, 512], F32, tag="lg", name="bcp")
                            nc.tensor.matmul(
                                bcp[:], ones_sb[:], rec3.pop((h2, qb))[0:1, :],
                                start=True, stop=True,
                            )
                            nc.vector.tensor_mul(
                                vn_all[64 * h2:64 * (h2 + 1),
                                       S * j3 + 512 * qb:S * j3 + 512 * (qb + 1)],
                                va_tiles.pop(8 * j3 + 4 * h2 + qb)[:],
                                bcp[:],
                            )
                        for tokb in range(4 * qb, 4 * qb + 4):
                            o_sb = obp.tile([128, HID], F32)
                            for ob in range(2):
                                opp = psP.tile([128, 512], F32, tag="pp")
                                nc.tensor.matmul(
                                    opp[:],
                                    vn3[:, j3, 128 * tokb:128 * (tokb + 1)],
                                    wo_sb[j3][:, 512 * ob:512 * (ob + 1)],
                                    start=True, stop=True,
                                )
                                nc.vector.tensor_add(
                                    o_sb[:, 512 * ob:512 * (ob + 1)], opp[:],
                                    op_tiles.pop((tokb, ob))[:],
                                )
                            nc.sync.dma_start(o[128 * tokb:128 * (tokb + 1), :], o_sb[:])

                    def pair3():
                        for qb in range(QB):
                            for h2 in range(2):
                                den = nrm3.tile([1, 512], BF16, tag="dflat")
                                attn_block(j3, h2, qb, kt_sb3, qt_t[j3 % 2][h2],
                                           den[0:1, :])
                                den3[(h2, qb)] = den
                            partial_chunk(qb)
                            if qb >= 1:
                                chain3(0, qb - 1)
                                chain3(1, qb - 1)
                            if qb >= 2:
                                finish(qb - 2)
                        chain3(0, QB - 1)
                        chain3(1, QB - 1)
                        finish(QB - 2)
                        finish(QB - 1)

                    pair3()

        if n_iter > 1:
            with tc.For_i(0, n_iter, 1):
                body()
        else:
            body()

    nc.compile()
    return nc


def shard_inputs(x, w_qkv, w_o):
    x = np.asarray(x, dtype=np.float32)
    w_qkv = np.asarray(w_qkv, dtype=np.float32)
    w_o = np.asarray(w_o, dtype=np.float32)
    import ml_dtypes
    bf = ml_dtypes.bfloat16

    # w_qkv row (h*192 + c): c<64 q, 64<=c<128 k, 128<=c<192 v
    w3 = w_qkv.reshape(H, 3 * D, HID)
    wq_h = w3[:, 0:D, :]        # [H, D, HID]
    wk_h = w3[:, D:2 * D, :]
    wv_h = w3[:, 2 * D:3 * D, :]
    wo_t = w_o.T                # [HID(vals feat, h-major), HID(out)]

    cone8 = np.ones((128, 512), np.float32).astype(bf)
    cone = np.ones((1, 64), np.float32).astype(bf)
    in_maps = []
    for core in range(N_CORES):
        b, g = core // G, core % G
        hsel = slice(HG * g, HG * (g + 1))
        wq_g = wq_h[hsel].reshape(NPAIR, 2 * D, HID).transpose(0, 2, 1).reshape(NPAIR * HID, 128)
        wk_g = wk_h[hsel].reshape(NPAIR, 2 * D, HID).transpose(0, 2, 1).reshape(NPAIR * HID, 128)
        wv_g = wv_h[hsel].reshape(HG * D, HID).T        # [HID, 512]
        wo_g = wo_t[HG * D * g:HG * D * (g + 1), :]     # [512, HID]
        in_maps.append({
            "xt": np.ascontiguousarray(x[b].T).astype(bf),
            "wq": np.ascontiguousarray(wq_g).astype(bf),
            "wk": np.ascontiguousarray(wk_g).astype(bf),
            "wv": np.ascontiguousarray(wv_g).astype(bf),
            "wo": np.ascontiguousarray(wo_g).astype(bf),
            "cone8": cone8, "cone": cone,
        })
    return in_maps


_NC_CACHE = {}


def get_nc(n_iter: int = 1):
    if n_iter not in _NC_CACHE:
        _NC_CACHE[n_iter] = build_nc(n_iter)
    return _NC_CACHE[n_iter]


def kernel(x, w_qkv, w_o):
    nc = get_nc(1)
    in_maps = shard_inputs(x, w_qkv, w_o)
    res = run_bass_kernel_spmd(nc, in_maps, list(range(N_CORES)))
    out = np.empty((B, S, HID), np.float32)
    for b in range(B):
        out[b] = res.results[G * b]["o"]
        for g in range(1, G):
            out[b] += res.results[G * b + g]["o"]
    return out



# revision 28
# speedup vs baseline: 1.0388x; 1.0388x over previous
"""Multi-head attention (B=4, S=2048, HID=1024, H=16, D=64) on 8 trn2 cores.

Sharding: batch x head-group (4 x 2). Core (2b+g) owns batch b and heads
8g..8g+7 over the FULL sequence: Q/K/V projections for its 8 heads,
attention, and a partial o-projection over its 512 value features. The host
sums the two partial o outputs per batch (the "all-reduce after o_proj"
done host-side) -- no duplicated projection work, no collectives.

Per-core dataflow (all matmuls full 128-partition moving operands, bf16,
fp32 PSUM accumulate -- avoids the half-bandwidth 64-partition moving path
and PE tiling-mode-switch drains):
  - K.T per pair packed [128=2x64 feat, token] bf16
  - Q.T per head zero-padded to [128, token] bf16 (other head's rows = 0),
    so logits contract over 128 partitions with the packed K stationary
  - V' in [token, (kt, head, 65)] bf16 with a ones column per head
    (softmax denominator falls out of the AV matmul as row 64)
  - logits L.T[k, q] in PSUM [128, 1024] (2 k-tiles); exp on ScalarE
  - AV accumulates vals'[65, 512] over 16 k-tiles; row 64 = denominator
  - denominator rows DMA'd from PSUM into a partition-major [8, 512] tile;
    ONE reciprocal per pair (free-size bound: 8x cheaper than reciprocal of
    broadcast tiles); bounced via DRAM back to a flat row, PE-broadcast,
    DVE multiply into vn
  - o_proj tail: vn (bf16) @ w_o.T shard (bf16) over 4 feature chunks

Schedule (this session's changes):
  - startup: ones constant loads first and feeds ~5us of PE clock-warmup
    junk; x loads in token-block strips so the pair-0 projection streams
    behind the DMA instead of waiting for the whole 4MB
  - pair 3 is pipelined per query-block: both heads' attention for qb,
    o-proj partial chunks 0-2 for qb's tokens, ACT-based reciprocal chains
    (1/x = exp(-ln x)) one qb late, then normalize + chunk-3 + add + store
    for qb-2; the output DMA streams during pair-3 attention
"""
import contextlib
import sys
sys.path.insert(0, "/opt/trn_rl_repo")
import numpy as np

import concourse.bass as bass
import concourse.mybir as mybir
import concourse.tile as tile
from concourse import bacc
from concourse.bass_utils import run_bass_kernel_spmd

F32 = mybir.dt.float32
F32R = mybir.dt.float32r
BF16 = mybir.dt.bfloat16
EXP = mybir.ActivationFunctionType.Exp

B, S, HID, H, D = 4, 2048, 1024, 16, 64
G = 2                  # head groups (cores per batch)
HG = H // G            # 8 heads per core
NPAIR = HG // 2        # 4 head pairs per core
HT = HID // 128        # 8 hid contraction tiles
TB = S // 512          # 4 proj token blocks
KT = S // 128          # 16 key-token tiles
QB = S // 512          # 4 query blocks of 512
N_CORES = 8


def build_nc(n_iter: int = 1):
    nc = bacc.Bacc(None, target_bir_lowering=False)

    # all inputs pre-swizzled host-side to [128, ...] partition-major so
    # every DMA is a contiguous-line 2D copy (see shard_inputs)
    xt = nc.dram_tensor("xt", [128, HT * S], BF16, kind="ExternalInput")
    wq = nc.dram_tensor("wq", [128, NPAIR * HID], BF16, kind="ExternalInput")
    wk = nc.dram_tensor("wk", [128, NPAIR * HID], BF16, kind="ExternalInput")
    wv = nc.dram_tensor("wv", [128, HT * HG * D], BF16, kind="ExternalInput")
    wo = nc.dram_tensor("wo", [128, NPAIR * HID], BF16, kind="ExternalInput")
    cone8 = nc.dram_tensor("cone8", [128, 512], BF16, kind="ExternalInput")
    cone = nc.dram_tensor("cone", [1, 64], BF16, kind="ExternalInput")
    o = nc.dram_tensor("o", [S, HID], F32, kind="ExternalOutput")

    with tile.TileContext(nc) as tc:
        def body():
            with contextlib.ExitStack() as _st:
                constp = _st.enter_context(tc.tile_pool(name="const", bufs=1))
                xtp = _st.enter_context(tc.tile_pool(name="xtp", bufs=1))
                vtp = _st.enter_context(tc.tile_pool(name="vtp", bufs=1))
                vnp = _st.enter_context(tc.tile_pool(name="vnp", bufs=1))
                wop = _st.enter_context(tc.tile_pool(name="wop", bufs=1))
                ktqp = _st.enter_context(tc.tile_pool(name="ktqp", bufs=1))
                opsb = _st.enter_context(tc.tile_pool(name="opsb", bufs=28))
                # ones constant doubles as PE clock-warmup fodder: it is the
                # FIRST dma (128KB, lands ~1us) so the junk matmuls below can
                # warm the HAM clock gate while the 5.8MB of real inputs load
                ones8_sb = constp.tile([128, 512], BF16)
                nc.sync.dma_start(ones8_sb[:], cone8[:])
                ones_sb = constp.tile([1, 64], BF16)
                nc.sync.dma_start(ones_sb[:], cone[:])

                # x resident in SBUF, TOKEN-BLOCK-major: strip tb holds hid
                # tiles 0-7 for tokens [512*tb, 512*(tb+1)) contiguously, so
                # each strip's DMA write range is exact (no false deps) and
                # the pair-0 projection streams behind the x load strip by
                # strip instead of waiting for the whole 4MB.
                xt_all = xtp.tile([128, HT * S], BF16, name="xtall")

                def xt_at(ht, tok, width):
                    half, off = divmod(tok, 1024)
                    assert off + width <= 1024
                    base = HT * 1024 * half + 1024 * ht + off
                    return xt_all[:, base:base + width]

                wo_all = wop.tile([128, NPAIR * HID], BF16, name="woall")
                wo_sb = [wo_all[:, HID * c:HID * (c + 1)] for c in range(NPAIR)]

                # V' [token, (kt, head, 65)] bf16, resident in SBUF
                vt = vtp.tile([128, KT * HG * 65], BF16)
                vt4 = vt.rearrange("p (t h c) -> p t h c", h=HG, c=65)
                # normalized values [feat(128=2 heads), pair-chunk, token]
                vn_all = vnp.tile([128, NPAIR * S], BF16)

                # persistent K/Q tiles, double-buffered across pairs.
                # qt_h zero-halves are memset once and never overwritten.
                kt_t = [ktqp.tile([128, S], BF16, name=f"kt{i}") for i in range(2)]
                qt_t = [[ktqp.tile([128, S], BF16, name=f"qt{i}{h2}") for h2 in range(2)]
                        for i in range(2)]
                for i in range(2):
                    nc.any.memset(qt_t[i][0][64:128, :], 0.0)
                    nc.any.memset(qt_t[i][1][0:64, :], 0.0)

                with contextlib.ExitStack() as _st2:
                    wvp = _st2.enter_context(tc.tile_pool(name="wvp", bufs=1))
                    wkp = _st2.enter_context(tc.tile_pool(name="wkp", bufs=2))
                    wqp = _st2.enter_context(tc.tile_pool(name="wqp", bufs=2))
                    ptp = _st2.enter_context(tc.tile_pool(name="ptp", bufs=3))
                    vap = _st2.enter_context(tc.tile_pool(name="vap", bufs=18))
                    nrm = _st2.enter_context(tc.tile_pool(name="nrm", bufs=2))
                    nrm3 = _st2.enter_context(tc.tile_pool(name="nrm3", bufs=4))
                    rfp = _st2.enter_context(tc.tile_pool(name="rfp", bufs=2))
                    obp = _st2.enter_context(tc.tile_pool(name="obp", bufs=2))
                    dramp = _st2.enter_context(tc.tile_pool(name="dramp", bufs=3, space="DRAM"))
                    psP = _st2.enter_context(tc.tile_pool(name="psP", bufs=2, space="PSUM"))
                    psL = _st2.enter_context(tc.tile_pool(name="psL", bufs=2, space="PSUM"))
                    psV = _st2.enter_context(tc.tile_pool(name="psV", bufs=2, space="PSUM"))
                    def proj_dma(j, w_dram, pool):
                        w_p = pool.tile([128, HID], BF16, tag="wp")
                        nc.sync.dma_start(w_p[:], w_dram[:, HID * j:HID * (j + 1)])
                        return w_p

                    # DMA queue order = emission order: pair-0 weights, then x
                    # in two token-half strips so the pair-0 projection (which
                    # consumes x token-block by token-block) starts after
                    # ~2.5MB instead of waiting for the whole 4MB x load.
                    wk0 = proj_dma(0, wk, wkp)
                    wq0 = proj_dma(0, wq, wqp)
                    for half in range(2):
                        nc.sync.dma_start(
                            xt_all[:, HT * 1024 * half:HT * 1024 * (half + 1)],
                            xt[:, HT * 1024 * half:HT * 1024 * (half + 1)],
                        )
                    wv_all = wvp.tile([128, HT * HG * D], BF16, name="wvall")
                    wv_sb = [wv_all[:, HG * D * t:HG * D * (t + 1)] for t in range(HT)]
                    nc.sync.dma_start(wv_all[:], wv[:])
                    nc.sync.dma_start(wo_all[:], wo[:])

                    # prewarm the activation tables while input DMAs are in
                    # flight -- Ln first pins the natural_log_exp_and_others
                    # set, which also serves every Exp (no mid-kernel reload)
                    warm = nrm.tile([1, 8], BF16, tag="warm")
                    nc.scalar.activation(warm[:], ones8_sb[0:1, 0:8],
                                         mybir.ActivationFunctionType.Ln)
                    nc.scalar.activation(warm[:], ones8_sb[0:1, 0:8], EXP, scale=0.0)
                    # warm the PE HAM clock gate on the ones constant (lands
                    # ~1us, far ahead of x): ~6us of junk matmuls so the first
                    # projection runs at 2.4GHz, fully overlapped with the
                    # x/wv/wo input DMAs
                    for _ in range(12):
                        jps = psP.tile([128, 512], F32, tag="pp")
                        nc.tensor.matmul(
                            jps[:], ones8_sb[:, 0:128], ones8_sb[:],
                            start=True, stop=True,
                        )

                    def v_proj(tokt):
                        vps = psP.tile([128, 512], F32, tag="pp")
                        for ht in range(HT):
                            nc.tensor.matmul(
                                vps[:],
                                xt_at(ht, 128 * tokt, 128),
                                wv_sb[ht][:],
                                start=(ht == 0), stop=(ht == HT - 1),
                            )
                        nc.vector.tensor_copy(
                            vt4[:, tokt, :, 0:64],
                            vps.rearrange("p (h c) -> p h c", c=64),
                        )
                        nc.vector.tensor_copy(vt4[:, tokt, :, 64], ones8_sb[:, 0:8])

                    # ---- pair pipeline: K/Q proj + attention + normalize ----
                    va_tiles = {}

                    def proj_pair(w_p, evac):
                        for tb in range(TB):
                            pps = psP.tile([128, 512], F32, tag="pp")
                            for ht in range(HT):
                                nc.tensor.matmul(
                                    pps[:],
                                    w_p[:, 128 * ht:128 * (ht + 1)],
                                    xt_at(ht, 512 * tb, 512),
                                    start=(ht == 0), stop=(ht == HT - 1),
                                )
                            evac(tb, pps)

                    def attn_block(j, h2, qb, kt_sb, qt_sb, den_out, v_inter=False):
                        h = 2 * j + h2
                        first_mm = None
                        vals = psV.tile([65, 512], F32, tag="vv")
                        for ktp2 in range(KT // 2):
                            lg = psL.tile([128, 1024], F32, tag="lg")
                            for u in range(2):
                                kt = 2 * ktp2 + u
                                mm = nc.tensor.matmul(
                                    lg[:, 512 * u:512 * (u + 1)],
                                    kt_sb[:, 128 * kt:128 * (kt + 1)],
                                    qt_sb[:, 512 * qb:512 * (qb + 1)],
                                    start=True, stop=True,
                                )
                                if first_mm is None:
                                    first_mm = mm
                            pt = ptp.tile([128, 1024], BF16, tag="pt")
                            nc.scalar.activation(pt[:], lg[:], EXP, scale=0.125)
                            if v_inter:
                                # first attention block: V' projection for these
                                # two k-tiles lands just ahead of their AV use
                                v_proj(2 * ktp2)
                                v_proj(2 * ktp2 + 1)
                            for u in range(2):
                                kt = 2 * ktp2 + u
                                nc.tensor.matmul(
                                    vals[:],
                                    vt4[:, kt, h, :],
                                    pt[:, 512 * u:512 * (u + 1)],
                                    start=(ktp2 == 0 and u == 0),
                                    stop=(ktp2 == KT // 2 - 1 and u == 1),
                                )
                        nc.vector.tensor_copy(den_out, vals[64:65, :])
                        va = vap.tile([64, 512], BF16, tag="va")
                        nc.vector.tensor_copy(va[:], vals[0:64, :])
                        va_tiles[8 * j + 4 * h2 + qb] = va
                        return first_mm

                    rec_drams = {}
                    rec_flats = {}

                    def chain(j, h2, den_flat, use_act=False):
                        # reciprocal of this half-pair's 4 denominator rows.
                        # engine ops cannot address partitions 1..31, so either
                        # bounce through DRAM to partition-major for the DVE
                        # reciprocal, or (for the last chain, when the scalar
                        # engine has gone idle) reciprocal the flat row on ACT.
                        if use_act:
                            # 1/x = exp(-ln x): two ACT ops on the flat row --
                            # the scalar engine is idle by the last chain and
                            # this skips two DMA bounce hops on the o-proj
                            # critical path (both fns live in the preloaded
                            # natural_log_exp table set)
                            lnt = nrm.tile([1, 4 * 512], F32, tag="lnt")
                            nc.scalar.activation(lnt[:], den_flat[:],
                                                 mybir.ActivationFunctionType.Ln)
                            rec_flat = nrm.tile([1, 4 * 512], BF16, tag="rflat")
                            nc.scalar.activation(rec_flat[:], lnt[:], EXP, scale=-1.0)
                            rec_flats[(j, h2)] = rec_flat
                        else:
                            rec_dram = dramp.tile([4, 512], BF16, tag="rdram")
                            den_dram = dramp.tile([4, 512], BF16, tag="ddram")
                            nc.sync.dma_start(
                                den_dram.rearrange("r c -> (r c)")[None, :], den_flat[0:1, :]
                            )
                            den_sq = nrm.tile([4, 512], BF16, tag="dsq")
                            nc.sync.dma_start(den_sq[:], den_dram[:])
                            rec_sq = nrm.tile([4, 512], BF16, tag="rsq")
                            with nc.allow_low_precision(reason="denominator reciprocal in bf16"):
                                nc.vector.reciprocal(rec_sq[:], den_sq[:])
                            nc.sync.dma_start(rec_dram[:], rec_sq[:])
                            rec_drams[(j, h2)] = rec_dram

                    def normalize(j, h2):
                        # runs a half-pair or more late: the reciprocal chain
                        # has had a full attention half to complete, so nothing
                        # here blocks the in-order engine streams. The last
                        # chain instead broadcasts via a PE matmul straight
                        # from the flat reciprocal row (PE is idle by then and
                        # this skips two DMA hops on the o-proj critical path).
                        rec_flat = rec_flats.pop((j, h2), None)
                        rec_dram = None if rec_flat is not None else rec_drams.pop((j, h2))
                        for qb in range(QB):
                            if rec_flat is not None:
                                bcp = psL.tile([64, 512], F32, tag="lg", name="bcp")
                                nc.tensor.matmul(
                                    bcp[:], ones_sb[:],
                                    rec_flat[0:1, 512 * qb:512 * (qb + 1)],
                                    start=True, stop=True,
                                )
                                bop = bcp
                            else:
                                bcs = nrm.tile([64, 512], BF16, tag="bcs")
                                nc.sync.dma_start(
                                    bcs[:], rec_dram[qb:qb + 1, :].broadcast_to([64, 512])
                                )
                                bop = bcs
                            nc.vector.tensor_mul(
                                vn_all[64 * h2:64 * (h2 + 1),
                                       S * j + 512 * qb:S * j + 512 * (qb + 1)],
                                va_tiles.pop(8 * j + 4 * h2 + qb)[:],
                                bop[:],
                            )

                    def k_evac_f(kt_sb):
                        def k_evac(tb, pps):
                            nc.vector.tensor_copy(kt_sb[:, 512 * tb:512 * (tb + 1)], pps[:])
                        return k_evac

                    def q_evac_f(jj):
                        def q_evac(tb, pps):
                            nc.vector.tensor_copy(
                                qt_t[jj % 2][0][0:64, 512 * tb:512 * (tb + 1)], pps[0:64, :])
                            nc.vector.tensor_copy(
                                qt_t[jj % 2][1][64:128, 512 * tb:512 * (tb + 1)], pps[64:128, :])
                        return q_evac

                    proj_pair(wk0, k_evac_f(kt_t[0]))
                    proj_pair(wq0, q_evac_f(0))
                    for j in range(NPAIR - 1):
                        kt_sb = kt_t[j % 2]
                        for h2 in range(2):
                            den_flat = nrm.tile([1, 4 * 512], BF16, tag="dflat")
                            for qb in range(QB):
                                attn_block(j, h2, qb, kt_sb, qt_t[j % 2][h2],
                                           den_flat[0:1, 512 * qb:512 * (qb + 1)],
                                           v_inter=(j == 0 and h2 == 0 and qb == 0))
                            chain(j, h2, den_flat, use_act=False)
                        normalize(j, 0)
                        proj_pair(proj_dma(j + 1, wk, wkp), k_evac_f(kt_t[(j + 1) % 2]))
                        proj_pair(proj_dma(j + 1, wq, wqp), q_evac_f(j + 1))
                        normalize(j, 1)

                    # ---- pair 3, pipelined per query-block ----
                    # No projections remain to fill PE slack, so the o-proj
                    # partials (chunks 0-2, which need only pairs 0-2's vn)
                    # interleave per-qb here, and each qb's reciprocal chain /
                    # normalize / chunk-3 / add / store overlaps the NEXT qb's
                    # ACT-paced attention instead of serializing at the end.
                    j3 = NPAIR - 1
                    kt_sb3 = kt_t[j3 % 2]
                    vn3 = vn_all.rearrange("p (c s) -> p c s", c=NPAIR)
                    op_tiles = {}
                    rec3 = {}
                    den3 = {}

                    def partial_chunk(qb):
                        for tokb in range(4 * qb, 4 * qb + 4):
                            for ob in range(2):
                                opp = psP.tile([128, 512], F32, tag="pp")
                                for c in range(NPAIR - 1):
                                    nc.tensor.matmul(
                                        opp[:],
                                        vn3[:, c, 128 * tokb:128 * (tokb + 1)],
                                        wo_sb[c][:, 512 * ob:512 * (ob + 1)],
                                        start=(c == 0), stop=(c == NPAIR - 2),
                                    )
                                op_sb = opsb.tile([128, 512], BF16, tag="op")
                                nc.vector.tensor_copy(op_sb[:], opp[:])
                                op_tiles[(tokb, ob)] = op_sb

                    def chain3(h2, qb):
                        # single-op DVE approximate reciprocal (~18 bits, no
                        # DRAM bounce). NOT Ln+Exp on ACT: Ln lives in a
                        # different activation-table set than the exp stream's,
                        # so a mid-kernel Ln forces two 1.3us ACT_TABLE_LOADs
                        # and stalls the exp pipeline (measured +10us).
                        den = den3.pop((h2, qb))
                        rec_f = rfp.tile([1, 512], F32, tag="recf")
                        nc.vector.reciprocal_approx_fast(rec_f[:], den[0:1, :])
                        rec_flat = nrm3.tile([1, 512], BF16, tag="rflat")
                        nc.vector.tensor_copy(rec_flat[:], rec_f[:])
                        rec3[(h2, qb)] = rec_flat

                    def finish(qb):
                        # normalize both halves for this qb (PE row-broadcast
                        # of the reciprocal + DVE mul), then chunk-3 o-proj,
                        # add the banked partial, and store this token range
                        for h2 in range(2):
                            bcp = psL.tile([64, 512], F32, tag="lg", name="bcp")
                            nc.tensor.matmul(
                                bcp[:], ones_sb[:], rec3.pop((h2, qb))[0:1, :],
                                start=True, stop=True,
                            )
                            nc.vector.tensor_mul(
                                vn_all[64 * h2:64 * (h2 + 1),
                                       S * j3 + 512 * qb:S * j3 + 512 * (qb + 1)],
                                va_tiles.pop(8 * j3 + 4 * h2 + qb)[:],
                                bcp[:],
                            )
                        for tokb in range(4 * qb, 4 * qb + 4):
                            o_sb = obp.tile([128, HID], F32)
                            for ob in range(2):
                                opp = psP.tile([128, 512], F32, tag="pp")
                                nc.tensor.matmul(
                                    opp[:],
                                    vn3[:, j3, 128 * tokb:128 * (tokb + 1)],
                                    wo_sb[j3][:, 512 * ob:512 * (ob + 1)],
                                    start=True, stop=True,
                                )
                                nc.vector.tensor_add(
                                    o_sb[:, 512 * ob:512 * (ob + 1)], opp[:],
                                    op_tiles.pop((tokb, ob))[:],
                                )
                            nc.sync.dma_start(o[128 * tokb:128 * (tokb + 1), :], o_sb[:])

                    def pair3():
                        for qb in range(QB):
                            for h2 in range(2):
                                den = nrm3.tile([1, 512], F32, tag="dflat")
                                attn_block(j3, h2, qb, kt_sb3, qt_t[j3 % 2][h2],
                                           den[0:1, :])
                                den3[(h2, qb)] = den
                            partial_chunk(qb)
                            if qb >= 1:
                                chain3(0, qb - 1)
                                chain3(1, qb - 1)
                            if qb >= 2:
                                finish(qb - 2)
                        chain3(0, QB - 1)
                        chain3(1, QB - 1)
                        finish(QB - 2)
                        finish(QB - 1)

                    pair3()

        if n_iter > 1:
            with tc.For_i(0, n_iter, 1):
                body()
        else:
            body()

    nc.compile()
    return nc


def shard_inputs(x, w_qkv, w_o):
    x = np.asarray(x, dtype=np.float32)
    w_qkv = np.asarray(w_qkv, dtype=np.float32)
    w_o = np.asarray(w_o, dtype=np.float32)
    import ml_dtypes
    bf = ml_dtypes.bfloat16

    # w_qkv row (h*192 + c): c<64 q, 64<=c<128 k, 128<=c<192 v
    w3 = w_qkv.reshape(H, 3 * D, HID)
    wq_h = w3[:, 0:D, :]        # [H, D, HID]
    wk_h = w3[:, D:2 * D, :]
    wv_h = w3[:, 2 * D:3 * D, :]
    wo_t = w_o.T                # [HID(vals feat, h-major), HID(out)]

    cone8 = np.ones((128, 512), np.float32).astype(bf)
    cone = np.ones((1, 64), np.float32).astype(bf)
    in_maps = []
    for core in range(N_CORES):
        b, g = core // G, core % G
        hsel = slice(HG * g, HG * (g + 1))
        # all weights are pre-swizzled host-side into [128-partition, ...]
        # layouts so every input DMA is a plain 2D copy with >=1KB
        # contiguous lines (the previous per-128-col rearrange DMAs moved
        # 256B lines and dominated the kernel's startup latency)
        wq_g = wq_h[hsel].reshape(NPAIR, 2 * D, HID).transpose(0, 2, 1).reshape(NPAIR * HID, 128)
        wk_g = wk_h[hsel].reshape(NPAIR, 2 * D, HID).transpose(0, 2, 1).reshape(NPAIR * HID, 128)
        wq_p = wq_g.reshape(NPAIR, HID // 128, 128, 128).transpose(2, 0, 1, 3).reshape(128, NPAIR * HID)
        wk_p = wk_g.reshape(NPAIR, HID // 128, 128, 128).transpose(2, 0, 1, 3).reshape(128, NPAIR * HID)
        wv_g = wv_h[hsel].reshape(HG * D, HID).T        # [HID, 512]
        wv_p = wv_g.reshape(HT, 128, HG * D).transpose(1, 0, 2).reshape(128, HT * HG * D)
        wo_g = wo_t[HG * D * g:HG * D * (g + 1), :]     # [512, HID]
        wo_p = wo_g.reshape(NPAIR, 128, HID).transpose(1, 0, 2).reshape(128, NPAIR * HID)
        xb = np.ascontiguousarray(x[b].T).astype(bf)    # [HID, S]
        # x in token-block-major half strips: [128, (half, hid-tile, 1024)]
        xs = xb.reshape(HT, 128, 2, 1024).transpose(1, 2, 0, 3).reshape(128, HT * S)
        in_maps.append({
            "xt": np.ascontiguousarray(xs),
            "wq": np.ascontiguousarray(wq_p).astype(bf),
            "wk": np.ascontiguousarray(wk_p).astype(bf),
            "wv": np.ascontiguousarray(wv_p).astype(bf),
            "wo": np.ascontiguousarray(wo_p).astype(bf),
            "cone8": cone8, "cone": cone,
        })
    return in_maps


_NC_CACHE = {}


def get_nc(n_iter: int = 1):
    if n_iter not in _NC_CACHE:
        _NC_CACHE[n_iter] = build_nc(n_iter)
    return _NC_CACHE[n_iter]


def kernel(x, w_qkv, w_o):
    nc = get_nc(1)
    in_maps = shard_inputs(x, w_qkv, w_o)
    res = run_bass_kernel_spmd(nc, in_maps, list(range(N_CORES)))
    out = np.empty((B, S, HID), np.float32)
    for b in range(B):
        out[b] = res.results[G * b]["o"]
        for g in range(1, G):
            out[b] += res.results[G * b + g]["o"]
    return out



# revision 32
# speedup vs baseline: 1.0699x; 1.0299x over previous
"""Multi-head attention (B=4, S=2048, HID=1024, H=16, D=64) on 8 trn2 cores.

Sharding: batch x head-group (4 x 2). Core (2b+g) owns batch b and heads
8g..8g+7 over the FULL sequence: Q/K/V projections for its 8 heads,
attention, and a partial o-projection over its 512 value features. The host
sums the two partial o outputs per batch (the "all-reduce after o_proj"
done host-side) -- no duplicated projection work, no collectives.

Per-core dataflow (all matmuls full 128-partition moving operands, bf16,
fp32 PSUM accumulate -- avoids the half-bandwidth 64-partition moving path
and PE tiling-mode-switch drains):
  - K.T per pair packed [128=2x64 feat, token] bf16
  - Q.T per head zero-padded to [128, token] bf16 (other head's rows = 0),
    so logits contract over 128 partitions with the packed K stationary
  - V' in [token, (kt, head, 65)] bf16 with a ones column per head
    (softmax denominator falls out of the AV matmul as row 64)
  - logits L.T[k, q] in PSUM [128, 1024] (2 k-tiles); exp on ScalarE
  - AV accumulates vals'[65, 512] over 16 k-tiles; row 64 = denominator
  - denominator rows DMA'd from PSUM into a partition-major [8, 512] tile;
    ONE reciprocal per pair (free-size bound: 8x cheaper than reciprocal of
    broadcast tiles); bounced via DRAM back to a flat row, PE-broadcast,
    DVE multiply into vn
  - o_proj tail: vn (bf16) @ w_o.T shard (bf16) over 4 feature chunks

Schedule (this session's changes):
  - startup: ones constant loads first and feeds ~5us of PE clock-warmup
    junk; x loads in token-block strips so the pair-0 projection streams
    behind the DMA instead of waiting for the whole 4MB
  - pair 3 is pipelined per query-block: both heads' attention for qb,
    o-proj partial chunks 0-2 for qb's tokens, ACT-based reciprocal chains
    (1/x = exp(-ln x)) one qb late, then normalize + chunk-3 + add + store
    for qb-2; the output DMA streams during pair-3 attention
"""
import contextlib
import sys
sys.path.insert(0, "/opt/trn_rl_repo")
import numpy as np

import concourse.bass as bass
import concourse.mybir as mybir
import concourse.tile as tile
from concourse import bacc
from concourse.bass_utils import run_bass_kernel_spmd

F32 = mybir.dt.float32
F32R = mybir.dt.float32r
BF16 = mybir.dt.bfloat16
EXP = mybir.ActivationFunctionType.Exp

B, S, HID, H, D = 4, 2048, 1024, 16, 64
G = 2                  # head groups (cores per batch)
HG = H // G            # 8 heads per core
NPAIR = HG // 2        # 4 head pairs per core
HT = HID // 128        # 8 hid contraction tiles
TB = S // 512          # 4 proj token blocks
KT = S // 128          # 16 key-token tiles
QB = S // 512          # 4 query blocks of 512
N_CORES = 8


def build_nc(n_iter: int = 1):
    nc = bacc.Bacc(None, target_bir_lowering=False)

    # all inputs pre-swizzled host-side to [128, ...] partition-major so
    # every DMA is a contiguous-line 2D copy (see shard_inputs)
    xt = nc.dram_tensor("xt", [128, HT * S], BF16, kind="ExternalInput")
    wq = nc.dram_tensor("wq", [128, NPAIR * HID], BF16, kind="ExternalInput")
    wk = nc.dram_tensor("wk", [128, NPAIR * HID], BF16, kind="ExternalInput")
    wv = nc.dram_tensor("wv", [128, HT * HG * D], BF16, kind="ExternalInput")
    wo = nc.dram_tensor("wo", [128, NPAIR * HID], BF16, kind="ExternalInput")
    cone8 = nc.dram_tensor("cone8", [128, 512], BF16, kind="ExternalInput")
    cone = nc.dram_tensor("cone", [1, 64], BF16, kind="ExternalInput")
    o = nc.dram_tensor("o", [S, HID], F32, kind="ExternalOutput")

    with tile.TileContext(nc) as tc:
        def body():
            with contextlib.ExitStack() as _st:
                constp = _st.enter_context(tc.tile_pool(name="const", bufs=1))
                xtp = _st.enter_context(tc.tile_pool(name="xtp", bufs=1))
                vtp = _st.enter_context(tc.tile_pool(name="vtp", bufs=1))
                vnp = _st.enter_context(tc.tile_pool(name="vnp", bufs=1))
                wop = _st.enter_context(tc.tile_pool(name="wop", bufs=1))
                ktqp = _st.enter_context(tc.tile_pool(name="ktqp", bufs=1))

                # ones constant doubles as PE clock-warmup fodder: it is the
                # FIRST dma (128KB, lands ~1us) so the junk matmuls below can
                # warm the HAM clock gate while the 5.8MB of real inputs load
                ones8_sb = constp.tile([128, 512], BF16)
                nc.sync.dma_start(ones8_sb[:], cone8[:])
                ones_sb = constp.tile([1, 64], BF16)
                nc.sync.dma_start(ones_sb[:], cone[:])

                # x resident in SBUF, TOKEN-BLOCK-major: strip tb holds hid
                # tiles 0-7 for tokens [512*tb, 512*(tb+1)) contiguously, so
                # each strip's DMA write range is exact (no false deps) and
                # the pair-0 projection streams behind the x load strip by
                # strip instead of waiting for the whole 4MB.
                xt_all = xtp.tile([128, HT * S], BF16, name="xtall")

                def xt_at(ht, tok, width):
                    half, off = divmod(tok, 1024)
                    assert off + width <= 1024
                    base = HT * 1024 * half + 1024 * ht + off
                    return xt_all[:, base:base + width]

                wo_all = wop.tile([128, NPAIR * HID], BF16, name="woall")
                wo_sb = [wo_all[:, HID * c:HID * (c + 1)] for c in range(NPAIR)]

                # V' [token, (kt, head, 65)] bf16, resident in SBUF
                vt = vtp.tile([128, KT * HG * 65], BF16)
                vt4 = vt.rearrange("p (t h c) -> p t h c", h=HG, c=65)
                # normalized values [feat(128=2 heads), pair-chunk, token]
                vn_all = vnp.tile([128, NPAIR * S], BF16)

                # persistent K/Q tiles, double-buffered across pairs.
                # qt_h zero-halves are memset once and never overwritten.
                kt_t = [ktqp.tile([128, S], BF16, name=f"kt{i}") for i in range(2)]
                qt_t = [[ktqp.tile([128, S], BF16, name=f"qt{i}{h2}") for h2 in range(2)]
                        for i in range(2)]
                for i in range(2):
                    nc.any.memset(qt_t[i][0][64:128, :], 0.0)
                    nc.any.memset(qt_t[i][1][0:64, :], 0.0)

                with contextlib.ExitStack() as _st2:
                    wvp = _st2.enter_context(tc.tile_pool(name="wvp", bufs=1))
                    wkp = _st2.enter_context(tc.tile_pool(name="wkp", bufs=2))
                    wqp = _st2.enter_context(tc.tile_pool(name="wqp", bufs=2))
                    ptp = _st2.enter_context(tc.tile_pool(name="ptp", bufs=3))
                    vap = _st2.enter_context(tc.tile_pool(name="vap", bufs=18))
                    nrm = _st2.enter_context(tc.tile_pool(name="nrm", bufs=2))
                    nrm3 = _st2.enter_context(tc.tile_pool(name="nrm3", bufs=4))
                    rfp = _st2.enter_context(tc.tile_pool(name="rfp", bufs=2))
                    obp = _st2.enter_context(tc.tile_pool(name="obp", bufs=3))
                    dramp = _st2.enter_context(tc.tile_pool(name="dramp", bufs=3, space="DRAM"))
                    psP = _st2.enter_context(tc.tile_pool(name="psP", bufs=2, space="PSUM"))
                    psL = _st2.enter_context(tc.tile_pool(name="psL", bufs=2, space="PSUM"))
                    psV = _st2.enter_context(tc.tile_pool(name="psV", bufs=2, space="PSUM"))
                    def proj_dma(j, w_dram, pool):
                        w_p = pool.tile([128, HID], BF16, tag="wp")
                        nc.sync.dma_start(w_p[:], w_dram[:, HID * j:HID * (j + 1)])
                        return w_p

                    # DMA queue order = emission order: pair-0 weights, then x
                    # in two token-half strips so the pair-0 projection (which
                    # consumes x token-block by token-block) starts after
                    # ~2.5MB instead of waiting for the whole 4MB x load.
                    wk0 = proj_dma(0, wk, wkp)
                    wq0 = proj_dma(0, wq, wqp)
                    for half in range(2):
                        nc.sync.dma_start(
                            xt_all[:, HT * 1024 * half:HT * 1024 * (half + 1)],
                            xt[:, HT * 1024 * half:HT * 1024 * (half + 1)],
                        )
                    wv_all = wvp.tile([128, HT * HG * D], BF16, name="wvall")
                    wv_sb = [wv_all[:, HG * D * t:HG * D * (t + 1)] for t in range(HT)]
                    nc.sync.dma_start(wv_all[:], wv[:])
                    nc.sync.dma_start(wo_all[:], wo[:])

                    # prewarm the activation tables while input DMAs are in
                    # flight -- Ln first pins the natural_log_exp_and_others
                    # set, which also serves every Exp (no mid-kernel reload)
                    warm = nrm.tile([1, 8], BF16, tag="warm")
                    nc.scalar.activation(warm[:], ones8_sb[0:1, 0:8],
                                         mybir.ActivationFunctionType.Ln)
                    nc.scalar.activation(warm[:], ones8_sb[0:1, 0:8], EXP, scale=0.0)
                    # warm the PE HAM clock gate on the ones constant (lands
                    # ~1us, far ahead of x): ~6us of junk matmuls so the first
                    # projection runs at 2.4GHz, fully overlapped with the
                    # x/wv/wo input DMAs
                    for _ in range(16):
                        jps = psP.tile([128, 512], F32, tag="pp")
                        nc.tensor.matmul(
                            jps[:], ones8_sb[:, 0:128], ones8_sb[:],
                            start=True, stop=True,
                        )

                    def v_proj(tokt):
                        vps = psP.tile([128, 512], F32, tag="pp")
                        for ht in range(HT):
                            nc.tensor.matmul(
                                vps[:],
                                xt_at(ht, 128 * tokt, 128),
                                wv_sb[ht][:],
                                start=(ht == 0), stop=(ht == HT - 1),
                            )
                        nc.vector.tensor_copy(
                            vt4[:, tokt, :, 0:64],
                            vps.rearrange("p (h c) -> p h c", c=64),
                        )
                        nc.vector.tensor_copy(vt4[:, tokt, :, 64], ones8_sb[:, 0:8])

                    # ---- pair pipeline: K/Q proj + attention + normalize ----
                    va_tiles = {}

                    def proj_pair(w_p, evac):
                        for tb in range(TB):
                            pps = psP.tile([128, 512], F32, tag="pp")
                            for ht in range(HT):
                                nc.tensor.matmul(
                                    pps[:],
                                    w_p[:, 128 * ht:128 * (ht + 1)],
                                    xt_at(ht, 512 * tb, 512),
                                    start=(ht == 0), stop=(ht == HT - 1),
                                )
                            evac(tb, pps)

                    def attn_block(j, h2, qb, kt_sb, qt_sb, den_out, v_inter=False):
                        h = 2 * j + h2
                        first_mm = None
                        vals = psV.tile([65, 512], F32, tag="vv")
                        for ktp2 in range(KT // 2):
                            lg = psL.tile([128, 1024], F32, tag="lg")
                            for u in range(2):
                                kt = 2 * ktp2 + u
                                mm = nc.tensor.matmul(
                                    lg[:, 512 * u:512 * (u + 1)],
                                    kt_sb[:, 128 * kt:128 * (kt + 1)],
                                    qt_sb[:, 512 * qb:512 * (qb + 1)],
                                    start=True, stop=True,
                                )
                                if first_mm is None:
                                    first_mm = mm
                            pt = ptp.tile([128, 1024], BF16, tag="pt")
                            nc.scalar.activation(pt[:], lg[:], EXP, scale=0.125)
                            if v_inter:
                                # first attention block: V' projection for these
                                # two k-tiles lands just ahead of their AV use
                                v_proj(2 * ktp2)
                                v_proj(2 * ktp2 + 1)
                            for u in range(2):
                                kt = 2 * ktp2 + u
                                nc.tensor.matmul(
                                    vals[:],
                                    vt4[:, kt, h, :],
                                    pt[:, 512 * u:512 * (u + 1)],
                                    start=(ktp2 == 0 and u == 0),
                                    stop=(ktp2 == KT // 2 - 1 and u == 1),
                                )
                        nc.vector.tensor_copy(den_out, vals[64:65, :])
                        va = vap.tile([64, 512], BF16, tag="va")
                        nc.vector.tensor_copy(va[:], vals[0:64, :])
                        va_tiles[8 * j + 4 * h2 + qb] = va
                        return first_mm

                    rec_drams = {}
                    rec_flats = {}

                    def chain(j, h2, den_flat, use_act=False):
                        # reciprocal of this half-pair's 4 denominator rows.
                        # engine ops cannot address partitions 1..31, so either
                        # bounce through DRAM to partition-major for the DVE
                        # reciprocal, or (for the last chain, when the scalar
                        # engine has gone idle) reciprocal the flat row on ACT.
                        if use_act:
                            # 1/x = exp(-ln x): two ACT ops on the flat row --
                            # the scalar engine is idle by the last chain and
                            # this skips two DMA bounce hops on the o-proj
                            # critical path (both fns live in the preloaded
                            # natural_log_exp table set)
                            lnt = nrm.tile([1, 4 * 512], F32, tag="lnt")
                            nc.scalar.activation(lnt[:], den_flat[:],
                                                 mybir.ActivationFunctionType.Ln)
                            rec_flat = nrm.tile([1, 4 * 512], BF16, tag="rflat")
                            nc.scalar.activation(rec_flat[:], lnt[:], EXP, scale=-1.0)
                            rec_flats[(j, h2)] = rec_flat
                        else:
                            rec_dram = dramp.tile([4, 512], BF16, tag="rdram")
                            den_dram = dramp.tile([4, 512], BF16, tag="ddram")
                            nc.sync.dma_start(
                                den_dram.rearrange("r c -> (r c)")[None, :], den_flat[0:1, :]
                            )
                            den_sq = nrm.tile([4, 512], BF16, tag="dsq")
                            nc.sync.dma_start(den_sq[:], den_dram[:])
                            rec_sq = nrm.tile([4, 512], BF16, tag="rsq")
                            with nc.allow_low_precision(reason="denominator reciprocal in bf16"):
                                nc.vector.reciprocal(rec_sq[:], den_sq[:])
                            nc.sync.dma_start(rec_dram[:], rec_sq[:])
                            rec_drams[(j, h2)] = rec_dram

                    def normalize(j, h2):
                        # runs a half-pair or more late: the reciprocal chain
                        # has had a full attention half to complete, so nothing
                        # here blocks the in-order engine streams. The last
                        # chain instead broadcasts via a PE matmul straight
                        # from the flat reciprocal row (PE is idle by then and
                        # this skips two DMA hops on the o-proj critical path).
                        rec_flat = rec_flats.pop((j, h2), None)
                        rec_dram = None if rec_flat is not None else rec_drams.pop((j, h2))
                        for qb in range(QB):
                            if rec_flat is not None:
                                bcp = psL.tile([64, 512], F32, tag="lg", name="bcp")
                                nc.tensor.matmul(
                                    bcp[:], ones_sb[:],
                                    rec_flat[0:1, 512 * qb:512 * (qb + 1)],
                                    start=True, stop=True,
                                )
                                bop = bcp
                            else:
                                bcs = nrm.tile([64, 512], BF16, tag="bcs")
                                nc.sync.dma_start(
                                    bcs[:], rec_dram[qb:qb + 1, :].broadcast_to([64, 512])
                                )
                                bop = bcs
                            nc.vector.tensor_mul(
                                vn_all[64 * h2:64 * (h2 + 1),
                                       S * j + 512 * qb:S * j + 512 * (qb + 1)],
                                va_tiles.pop(8 * j + 4 * h2 + qb)[:],
                                bop[:],
                            )

                    def k_evac_f(kt_sb):
                        def k_evac(tb, pps):
                            nc.vector.tensor_copy(kt_sb[:, 512 * tb:512 * (tb + 1)], pps[:])
                        return k_evac

                    def q_evac_f(jj):
                        def q_evac(tb, pps):
                            nc.vector.tensor_copy(
                                qt_t[jj % 2][0][0:64, 512 * tb:512 * (tb + 1)], pps[0:64, :])
                            nc.vector.tensor_copy(
                                qt_t[jj % 2][1][64:128, 512 * tb:512 * (tb + 1)], pps[64:128, :])
                        return q_evac

                    proj_pair(wk0, k_evac_f(kt_t[0]))
                    proj_pair(wq0, q_evac_f(0))
                    for j in range(NPAIR - 1):
                        kt_sb = kt_t[j % 2]
                        for h2 in range(2):
                            den_flat = nrm.tile([1, 4 * 512], BF16, tag="dflat")
                            for qb in range(QB):
                                attn_block(j, h2, qb, kt_sb, qt_t[j % 2][h2],
                                           den_flat[0:1, 512 * qb:512 * (qb + 1)],
                                           v_inter=(j == 0 and h2 == 0 and qb == 0))
                            chain(j, h2, den_flat, use_act=False)
                        normalize(j, 0)
                        proj_pair(proj_dma(j + 1, wk, wkp), k_evac_f(kt_t[(j + 1) % 2]))
                        proj_pair(proj_dma(j + 1, wq, wqp), q_evac_f(j + 1))
                        normalize(j, 1)

                    # ---- pair 3, pipelined per query-block ----
                    # No projections remain to fill PE slack, so the o-proj
                    # runs FUSED per token-block here (all 4 pair chunks in
                    # one PSUM accumulation, one f32 evacuation, one store) as
                    # soon as this qb's pair-3 values are normalized. 4
                    # matmuls per unit vs one DVE copy keeps the store stream
                    # PE-paced; each qb's store overlaps the next qb's
                    # attention, leaving only the last group's ~8us serial.
                    j3 = NPAIR - 1
                    kt_sb3 = kt_t[j3 % 2]
                    vn3 = vn_all.rearrange("p (c s) -> p c s", c=NPAIR)
                    rec3 = {}
                    den3 = {}

                    def chain3(h2, qb):
                        # single-op DVE approximate reciprocal (~18 bits, no
                        # DRAM bounce). NOT Ln+Exp on ACT: Ln lives in a
                        # different activation-table set than the exp stream's,
                        # so a mid-kernel Ln forces two 1.3us ACT_TABLE_LOADs
                        # and stalls the exp pipeline (measured +10us).
                        den = den3.pop((h2, qb))
                        rec_f = rfp.tile([1, 512], F32, tag="recf")
                        nc.vector.reciprocal_approx_fast(rec_f[:], den[0:1, :])
                        rec_flat = nrm3.tile([1, 512], BF16, tag="rflat")
                        nc.vector.tensor_copy(rec_flat[:], rec_f[:])
                        rec3[(h2, qb)] = rec_flat

                    def norm3(h2, qb):
                        # PE row-broadcast of the reciprocal + DVE mul
                        bcp = psL.tile([64, 512], F32, tag="lg", name="bcp")
                        nc.tensor.matmul(
                            bcp[:], ones_sb[:], rec3.pop((h2, qb))[0:1, :],
                            start=True, stop=True,
                        )
                        nc.vector.tensor_mul(
                            vn_all[64 * h2:64 * (h2 + 1),
                                   S * j3 + 512 * qb:S * j3 + 512 * (qb + 1)],
                            va_tiles.pop(8 * j3 + 4 * h2 + qb)[:],
                            bcp[:],
                        )

                    def store(qb):
                        for tokb in range(4 * qb, 4 * qb + 4):
                            o_sb = obp.tile([128, HID], F32)
                            for ob in range(2):
                                opp = psP.tile([128, 512], F32, tag="pp")
                                for c in range(NPAIR):
                                    nc.tensor.matmul(
                                        opp[:],
                                        vn3[:, c, 128 * tokb:128 * (tokb + 1)],
                                        wo_sb[c][:, 512 * ob:512 * (ob + 1)],
                                        start=(c == 0), stop=(c == NPAIR - 1),
                                    )
                                nc.vector.tensor_copy(
                                    o_sb[:, 512 * ob:512 * (ob + 1)], opp[:])
                            nc.sync.dma_start(o[128 * tokb:128 * (tokb + 1), :], o_sb[:])

                    def pair3():
                        for qb in range(QB):
                            for h2 in range(2):
                                den = nrm3.tile([1, 512], F32, tag="dflat")
                                attn_block(j3, h2, qb, kt_sb3, qt_t[j3 % 2][h2],
                                           den[0:1, :])
                                den3[(h2, qb)] = den
                            if qb >= 1:
                                store(qb - 1)
                            chain3(0, qb)
                            chain3(1, qb)
                            norm3(0, qb)
                            norm3(1, qb)
                        store(QB - 1)

                    pair3()

        if n_iter > 1:
            with tc.For_i(0, n_iter, 1):
                body()
        else:
            body()

    nc.compile()
    return nc


def shard_inputs(x, w_qkv, w_o):
    x = np.asarray(x, dtype=np.float32)
    w_qkv = np.asarray(w_qkv, dtype=np.float32)
    w_o = np.asarray(w_o, dtype=np.float32)
    import ml_dtypes
    bf = ml_dtypes.bfloat16

    # w_qkv row (h*192 + c): c<64 q, 64<=c<128 k, 128<=c<192 v
    w3 = w_qkv.reshape(H, 3 * D, HID)
    wq_h = w3[:, 0:D, :]        # [H, D, HID]
    wk_h = w3[:, D:2 * D, :]
    wv_h = w3[:, 2 * D:3 * D, :]
    wo_t = w_o.T                # [HID(vals feat, h-major), HID(out)]

    cone8 = np.ones((128, 512), np.float32).astype(bf)
    cone = np.ones((1, 64), np.float32).astype(bf)
    in_maps = []
    for core in range(N_CORES):
        b, g = core // G, core % G
        hsel = slice(HG * g, HG * (g + 1))
        # all weights are pre-swizzled host-side into [128-partition, ...]
        # layouts so every input DMA is a plain 2D copy with >=1KB
        # contiguous lines (the previous per-128-col rearrange DMAs moved
        # 256B lines and dominated the kernel's startup latency)
        wq_g = wq_h[hsel].reshape(NPAIR, 2 * D, HID).transpose(0, 2, 1).reshape(NPAIR * HID, 128)
        wk_g = wk_h[hsel].reshape(NPAIR, 2 * D, HID).transpose(0, 2, 1).reshape(NPAIR * HID, 128)
        wq_p = wq_g.reshape(NPAIR, HID // 128, 128, 128).transpose(2, 0, 1, 3).reshape(128, NPAIR * HID)
        wk_p = wk_g.reshape(NPAIR, HID // 128, 128, 128).transpose(2, 0, 1, 3).reshape(128, NPAIR * HID)
        wv_g = wv_h[hsel].reshape(HG * D, HID).T        # [HID, 512]
        wv_p = wv_g.reshape(HT, 128, HG * D).transpose(1, 0, 2).reshape(128, HT * HG * D)
        wo_g = wo_t[HG * D * g:HG * D * (g + 1), :]     # [512, HID]
        wo_p = wo_g.reshape(NPAIR, 128, HID).transpose(1, 0, 2).reshape(128, NPAIR * HID)
        xb = np.ascontiguousarray(x[b].T).astype(bf)    # [HID, S]
        # x in token-block-major half strips: [128, (half, hid-tile, 1024)]
        xs = xb.reshape(HT, 128, 2, 1024).transpose(1, 2, 0, 3).reshape(128, HT * S)
        in_maps.append({
            "xt": np.ascontiguousarray(xs),
            "wq": np.ascontiguousarray(wq_p).astype(bf),
            "wk": np.ascontiguousarray(wk_p).astype(bf),
            "wv": np.ascontiguousarray(wv_p).astype(bf),
            "wo": np.ascontiguousarray(wo_p).astype(bf),
            "cone8": cone8, "cone": cone,
        })
    return in_maps


_NC_CACHE = {}


def get_nc(n_iter: int = 1):
    if n_iter not in _NC_CACHE:
        _NC_CACHE[n_iter] = build_nc(n_iter)
    return _NC_CACHE[n_iter]


def kernel(x, w_qkv, w_o):
    nc = get_nc(1)
    in_maps = shard_inputs(x, w_qkv, w_o)
    res = run_bass_kernel_spmd(nc, in_maps, list(range(N_CORES)))
    out = np.empty((B, S, HID), np.float32)
    for b in range(B):
        out[b] = res.results[G * b]["o"]
        for g in range(1, G):
            out[b] += res.results[G * b + g]["o"]
    return out



# revision 37
# speedup vs baseline: 1.0803x; 1.0096x over previous
"""Multi-head attention (B=4, S=2048, HID=1024, H=16, D=64) on 8 trn2 cores.

Sharding: batch x head-group (4 x 2). Core (2b+g) owns batch b and heads
8g..8g+7 over the FULL sequence: Q/K/V projections for its 8 heads,
attention, and a partial o-projection over its 512 value features. The host
sums the two partial o outputs per batch (the "all-reduce after o_proj"
done host-side) -- no duplicated projection work, no collectives.

Per-core dataflow (all matmuls full 128-partition moving operands, bf16,
fp32 PSUM accumulate -- avoids the half-bandwidth 64-partition moving path
and PE tiling-mode-switch drains):
  - K.T per pair packed [128=2x64 feat, token] bf16
  - Q.T per head zero-padded to [128, token] bf16 (other head's rows = 0),
    so logits contract over 128 partitions with the packed K stationary
  - V' in [token, (kt, head, 65)] bf16 with a ones column per head
    (softmax denominator falls out of the AV matmul as row 64)
  - logits L.T[k, q] in PSUM [128, 1024] (2 k-tiles); exp on ScalarE
  - AV accumulates vals'[65, 512] over 16 k-tiles; row 64 = denominator
  - denominator rows DMA'd from PSUM into a partition-major [8, 512] tile;
    ONE reciprocal per pair (free-size bound: 8x cheaper than reciprocal of
    broadcast tiles); bounced via DRAM back to a flat row, PE-broadcast,
    DVE multiply into vn
  - o_proj tail: vn (bf16) @ w_o.T shard (bf16) over 4 feature chunks

Schedule (this session's changes):
  - startup: ones constant loads first and feeds ~5us of PE clock-warmup
    junk; x loads in token-block strips so the pair-0 projection streams
    behind the DMA instead of waiting for the whole 4MB
  - pair 3 is pipelined per query-block: both heads' attention for qb,
    o-proj partial chunks 0-2 for qb's tokens, ACT-based reciprocal chains
    (1/x = exp(-ln x)) one qb late, then normalize + chunk-3 + add + store
    for qb-2; the output DMA streams during pair-3 attention
"""
import contextlib
import sys
sys.path.insert(0, "/opt/trn_rl_repo")
import numpy as np

import concourse.bass as bass
import concourse.mybir as mybir
import concourse.tile as tile
from concourse import bacc
from concourse.bass_utils import run_bass_kernel_spmd

F32 = mybir.dt.float32
F32R = mybir.dt.float32r
BF16 = mybir.dt.bfloat16
EXP = mybir.ActivationFunctionType.Exp

B, S, HID, H, D = 4, 2048, 1024, 16, 64
G = 2                  # head groups (cores per batch)
HG = H // G            # 8 heads per core
NPAIR = HG // 2        # 4 head pairs per core
HT = HID // 128        # 8 hid contraction tiles
TB = S // 512          # 4 proj token blocks
KT = S // 128          # 16 key-token tiles
QB = S // 512          # 4 query blocks of 512
N_CORES = 8


def build_nc(n_iter: int = 1):
    nc = bacc.Bacc(None, target_bir_lowering=False)

    # all inputs pre-swizzled host-side to [128, ...] partition-major so
    # every DMA is a contiguous-line 2D copy (see shard_inputs)
    xt = nc.dram_tensor("xt", [128, HT * S], BF16, kind="ExternalInput")
    wq = nc.dram_tensor("wq", [128, NPAIR * HID], BF16, kind="ExternalInput")
    wk = nc.dram_tensor("wk", [128, NPAIR * HID], BF16, kind="ExternalInput")
    wv = nc.dram_tensor("wv", [128, HT * HG * D], BF16, kind="ExternalInput")
    wo = nc.dram_tensor("wo", [128, NPAIR * HID], BF16, kind="ExternalInput")
    cone8 = nc.dram_tensor("cone8", [128, 512], BF16, kind="ExternalInput")
    cone = nc.dram_tensor("cone", [1, 64], BF16, kind="ExternalInput")
    o = nc.dram_tensor("o", [S, HID], F32, kind="ExternalOutput")

    with tile.TileContext(nc) as tc:
        def body():
            with contextlib.ExitStack() as _st:
                constp = _st.enter_context(tc.tile_pool(name="const", bufs=1))
                xtp = _st.enter_context(tc.tile_pool(name="xtp", bufs=1))
                vtp = _st.enter_context(tc.tile_pool(name="vtp", bufs=1))
                vnp = _st.enter_context(tc.tile_pool(name="vnp", bufs=1))
                wop = _st.enter_context(tc.tile_pool(name="wop", bufs=1))
                ktqp = _st.enter_context(tc.tile_pool(name="ktqp", bufs=1))

                # ones constant doubles as PE clock-warmup fodder: it is the
                # FIRST dma (128KB, lands ~1us) so the junk matmuls below can
                # warm the HAM clock gate while the 5.8MB of real inputs load
                ones8_sb = constp.tile([128, 512], BF16)
                nc.sync.dma_start(ones8_sb[:], cone8[:])
                ones_sb = constp.tile([1, 64], BF16)
                nc.sync.dma_start(ones_sb[:], cone[:])

                # x resident in SBUF, TOKEN-BLOCK-major: strip tb holds hid
                # tiles 0-7 for tokens [512*tb, 512*(tb+1)) contiguously, so
                # each strip's DMA write range is exact (no false deps) and
                # the pair-0 projection streams behind the x load strip by
                # strip instead of waiting for the whole 4MB.
                xt_all = xtp.tile([128, HT * S], BF16, name="xtall")

                def xt_at(ht, tok, width):
                    half, off = divmod(tok, 1024)
                    assert off + width <= 1024
                    base = HT * 1024 * half + 1024 * ht + off
                    return xt_all[:, base:base + width]

                wo_all = wop.tile([128, NPAIR * HID], BF16, name="woall")
                wo_sb = [wo_all[:, HID * c:HID * (c + 1)] for c in range(NPAIR)]

                # V' [token, (kt, head, 65)] bf16, resident in SBUF
                vt = vtp.tile([128, KT * HG * 65], BF16)
                vt4 = vt.rearrange("p (t h c) -> p t h c", h=HG, c=65)
                # normalized values [feat(128=2 heads), pair-chunk, token]
                vn_all = vnp.tile([128, NPAIR * S], BF16)

                # persistent K/Q tiles, double-buffered across pairs.
                # qt_h zero-halves are memset once and never overwritten.
                kt_t = [ktqp.tile([128, S], BF16, name=f"kt{i}") for i in range(2)]
                qt_t = [[ktqp.tile([128, S], BF16, name=f"qt{i}{h2}") for h2 in range(2)]
                        for i in range(2)]
                for i in range(2):
                    nc.any.memset(qt_t[i][0][64:128, :], 0.0)
                    nc.any.memset(qt_t[i][1][0:64, :], 0.0)

                with contextlib.ExitStack() as _st2:
                    wvp = _st2.enter_context(tc.tile_pool(name="wvp", bufs=1))
                    wkp = _st2.enter_context(tc.tile_pool(name="wkp", bufs=2))
                    wqp = _st2.enter_context(tc.tile_pool(name="wqp", bufs=2))
                    ptp = _st2.enter_context(tc.tile_pool(name="ptp", bufs=3))
                    vap = _st2.enter_context(tc.tile_pool(name="vap", bufs=18))
                    nrm = _st2.enter_context(tc.tile_pool(name="nrm", bufs=2))
                    nrm3 = _st2.enter_context(tc.tile_pool(name="nrm3", bufs=4))
                    rfp = _st2.enter_context(tc.tile_pool(name="rfp", bufs=2))
                    obp = _st2.enter_context(tc.tile_pool(name="obp", bufs=3))
                    dramp = _st2.enter_context(tc.tile_pool(name="dramp", bufs=3, space="DRAM"))
                    psP = _st2.enter_context(tc.tile_pool(name="psP", bufs=2, space="PSUM"))
                    psL = _st2.enter_context(tc.tile_pool(name="psL", bufs=2, space="PSUM"))
                    psV = _st2.enter_context(tc.tile_pool(name="psV", bufs=2, space="PSUM"))
                    def proj_dma(j, w_dram, pool):
                        w_p = pool.tile([128, HID], BF16, tag="wp")
                        nc.sync.dma_start(w_p[:], w_dram[:, HID * j:HID * (j + 1)])
                        return w_p

                    # DMA queue order = emission order: pair-0 weights, then x
                    # in two token-half strips so the pair-0 projection (which
                    # consumes x token-block by token-block) starts after
                    # ~2.5MB instead of waiting for the whole 4MB x load.
                    wk0 = proj_dma(0, wk, wkp)
                    nc.sync.dma_start(xt_all[:, 0:HT * 1024], xt[:, 0:HT * 1024])
                    wq0 = proj_dma(0, wq, wqp)
                    nc.sync.dma_start(xt_all[:, HT * 1024:], xt[:, HT * 1024:])
                    wv_all = wvp.tile([128, HT * HG * D], BF16, name="wvall")
                    wv_sb = [wv_all[:, HG * D * t:HG * D * (t + 1)] for t in range(HT)]
                    nc.sync.dma_start(wv_all[:], wv[:])
                    nc.sync.dma_start(wo_all[:], wo[:])

                    # prewarm the activation tables while input DMAs are in
                    # flight -- Ln first pins the natural_log_exp_and_others
                    # set, which also serves every Exp (no mid-kernel reload)
                    warm = nrm.tile([1, 8], BF16, tag="warm")
                    nc.scalar.activation(warm[:], ones8_sb[0:1, 0:8],
                                         mybir.ActivationFunctionType.Ln)
                    nc.scalar.activation(warm[:], ones8_sb[0:1, 0:8], EXP, scale=0.0)
                    # warm the PE HAM clock gate on the ones constant (lands
                    # ~1us, far ahead of x): junk matmuls sized to bridge the
                    # chip-HBM-bound input load (~12us until the first x half
                    # lands) so the first projection runs at 2.4GHz
                    for _ in range(36):
                        jps = psP.tile([128, 512], F32, tag="pp")
                        nc.tensor.matmul(
                            jps[:], ones8_sb[:, 0:128], ones8_sb[:],
                            start=True, stop=True,
                        )

                    def v_proj(tokt):
                        vps = psP.tile([128, 512], F32, tag="pp")
                        for ht in range(HT):
                            nc.tensor.matmul(
                                vps[:],
                                xt_at(ht, 128 * tokt, 128),
                                wv_sb[ht][:],
                                start=(ht == 0), stop=(ht == HT - 1),
                            )
                        nc.vector.tensor_copy(
                            vt4[:, tokt, :, 0:64],
                            vps.rearrange("p (h c) -> p h c", c=64),
                        )
                        nc.vector.tensor_copy(vt4[:, tokt, :, 64], ones8_sb[:, 0:8])

                    # ---- pair pipeline: K/Q proj + attention + normalize ----
                    va_tiles = {}

                    def proj_pair(w_p, evac, tbs=range(TB)):
                        for tb in tbs:
                            pps = psP.tile([128, 512], F32, tag="pp")
                            for ht in range(HT):
                                nc.tensor.matmul(
                                    pps[:],
                                    w_p[:, 128 * ht:128 * (ht + 1)],
                                    xt_at(ht, 512 * tb, 512),
                                    start=(ht == 0), stop=(ht == HT - 1),
                                )
                            evac(tb, pps)

                    def attn_block(j, h2, qb, kt_sb, qt_sb, den_out, v_inter=False):
                        h = 2 * j + h2
                        first_mm = None
                        vals = psV.tile([65, 512], F32, tag="vv")
                        for ktp2 in range(KT // 2):
                            lg = psL.tile([128, 1024], F32, tag="lg")
                            for u in range(2):
                                kt = 2 * ktp2 + u
                                mm = nc.tensor.matmul(
                                    lg[:, 512 * u:512 * (u + 1)],
                                    kt_sb[:, 128 * kt:128 * (kt + 1)],
                                    qt_sb[:, 512 * qb:512 * (qb + 1)],
                                    start=True, stop=True,
                                )
                                if first_mm is None:
                                    first_mm = mm
                            pt = ptp.tile([128, 1024], BF16, tag="pt")
                            nc.scalar.activation(pt[:], lg[:], EXP, scale=0.125)
                            if v_inter:
                                # first attention block: V' projection for these
                                # two k-tiles lands just ahead of their AV use
                                v_proj(2 * ktp2)
                                v_proj(2 * ktp2 + 1)
                            for u in range(2):
                                kt = 2 * ktp2 + u
                                nc.tensor.matmul(
                                    vals[:],
                                    vt4[:, kt, h, :],
                                    pt[:, 512 * u:512 * (u + 1)],
                                    start=(ktp2 == 0 and u == 0),
                                    stop=(ktp2 == KT // 2 - 1 and u == 1),
                                )
                        nc.vector.tensor_copy(den_out, vals[64:65, :])
                        va = vap.tile([64, 512], BF16, tag="va")
                        nc.vector.tensor_copy(va[:], vals[0:64, :])
                        va_tiles[8 * j + 4 * h2 + qb] = va
                        return first_mm

                    rec_drams = {}
                    rec_flats = {}

                    def chain(j, h2, den_flat, use_act=False):
                        # reciprocal of this half-pair's 4 denominator rows.
                        # engine ops cannot address partitions 1..31, so either
                        # bounce through DRAM to partition-major for the DVE
                        # reciprocal, or (for the last chain, when the scalar
                        # engine has gone idle) reciprocal the flat row on ACT.
                        if use_act:
                            # 1/x = exp(-ln x): two ACT ops on the flat row --
                            # the scalar engine is idle by the last chain and
                            # this skips two DMA bounce hops on the o-proj
                            # critical path (both fns live in the preloaded
                            # natural_log_exp table set)
                            lnt = nrm.tile([1, 4 * 512], F32, tag="lnt")
                            nc.scalar.activation(lnt[:], den_flat[:],
                                                 mybir.ActivationFunctionType.Ln)
                            rec_flat = nrm.tile([1, 4 * 512], BF16, tag="rflat")
                            nc.scalar.activation(rec_flat[:], lnt[:], EXP, scale=-1.0)
                            rec_flats[(j, h2)] = rec_flat
                        else:
                            rec_dram = dramp.tile([4, 512], BF16, tag="rdram")
                            den_dram = dramp.tile([4, 512], BF16, tag="ddram")
                            nc.sync.dma_start(
                                den_dram.rearrange("r c -> (r c)")[None, :], den_flat[0:1, :]
                            )
                            den_sq = nrm.tile([4, 512], BF16, tag="dsq")
                            nc.sync.dma_start(den_sq[:], den_dram[:])
                            rec_sq = nrm.tile([4, 512], BF16, tag="rsq")
                            with nc.allow_low_precision(reason="denominator reciprocal in bf16"):
                                nc.vector.reciprocal(rec_sq[:], den_sq[:])
                            nc.sync.dma_start(rec_dram[:], rec_sq[:])
                            rec_drams[(j, h2)] = rec_dram

                    def normalize(j, h2):
                        # runs a half-pair or more late: the reciprocal chain
                        # has had a full attention half to complete, so nothing
                        # here blocks the in-order engine streams. The last
                        # chain instead broadcasts via a PE matmul straight
                        # from the flat reciprocal row (PE is idle by then and
                        # this skips two DMA hops on the o-proj critical path).
                        rec_flat = rec_flats.pop((j, h2), None)
                        rec_dram = None if rec_flat is not None else rec_drams.pop((j, h2))
                        for qb in range(QB):
                            if rec_flat is not None:
                                bcp = psL.tile([64, 512], F32, tag="lg", name="bcp")
                                nc.tensor.matmul(
                                    bcp[:], ones_sb[:],
                                    rec_flat[0:1, 512 * qb:512 * (qb + 1)],
                                    start=True, stop=True,
                                )
                                bop = bcp
                            else:
                                bcs = nrm.tile([64, 512], BF16, tag="bcs")
                                nc.sync.dma_start(
                                    bcs[:], rec_dram[qb:qb + 1, :].broadcast_to([64, 512])
                                )
                                bop = bcs
                            nc.vector.tensor_mul(
                                vn_all[64 * h2:64 * (h2 + 1),
                                       S * j + 512 * qb:S * j + 512 * (qb + 1)],
                                va_tiles.pop(8 * j + 4 * h2 + qb)[:],
                                bop[:],
                            )

                    def k_evac_f(kt_sb):
                        def k_evac(tb, pps):
                            nc.vector.tensor_copy(kt_sb[:, 512 * tb:512 * (tb + 1)], pps[:])
                        return k_evac

                    def q_evac_f(jj):
                        def q_evac(tb, pps):
                            nc.vector.tensor_copy(
                                qt_t[jj % 2][0][0:64, 512 * tb:512 * (tb + 1)], pps[0:64, :])
                            nc.vector.tensor_copy(
                                qt_t[jj % 2][1][64:128, 512 * tb:512 * (tb + 1)], pps[64:128, :])
                        return q_evac

                    proj_pair(wk0, k_evac_f(kt_t[0]))
                    proj_pair(wq0, q_evac_f(0))
                    for j in range(NPAIR - 1):
                        kt_sb = kt_t[j % 2]
                        for h2 in range(2):
                            den_flat = nrm.tile([1, 4 * 512], BF16, tag="dflat")
                            for qb in range(QB):
                                attn_block(j, h2, qb, kt_sb, qt_t[j % 2][h2],
                                           den_flat[0:1, 512 * qb:512 * (qb + 1)],
                                           v_inter=(j == 0 and h2 == 0 and qb == 0))
                            chain(j, h2, den_flat, use_act=False)
                        normalize(j, 0)
                        proj_pair(proj_dma(j + 1, wk, wkp), k_evac_f(kt_t[(j + 1) % 2]))
                        wq_n = proj_dma(j + 1, wq, wqp)
                        if j < NPAIR - 2:
                            proj_pair(wq_n, q_evac_f(j + 1))
                        else:
                            # defer pair-3's Q token-block 3 into pair-3 qb0's
                            # window: that window is exp-paced with no store to
                            # fill PE slack (it's only needed by qb3's blocks)
                            proj_pair(wq_n, q_evac_f(j + 1), tbs=[0, 1, 2])
                            deferred_q3 = (wq_n, q_evac_f(j + 1))
                        normalize(j, 1)

                    # ---- pair 3, pipelined per query-block ----
                    # No projections remain to fill PE slack, so the o-proj
                    # runs FUSED per token-block here (all 4 pair chunks in
                    # one PSUM accumulation, one f32 evacuation, one store) as
                    # soon as this qb's pair-3 values are normalized. 4
                    # matmuls per unit vs one DVE copy keeps the store stream
                    # PE-paced; each qb's store overlaps the next qb's
                    # attention, leaving only the last group's ~8us serial.
                    j3 = NPAIR - 1
                    kt_sb3 = kt_t[j3 % 2]
                    vn3 = vn_all.rearrange("p (c s) -> p c s", c=NPAIR)
                    rec3 = {}
                    den3 = {}

                    def chain3(h2, qb):
                        # single-op DVE approximate reciprocal (~18 bits, no
                        # DRAM bounce). NOT Ln+Exp on ACT: Ln lives in a
                        # different activation-table set than the exp stream's,
                        # so a mid-kernel Ln forces two 1.3us ACT_TABLE_LOADs
                        # and stalls the exp pipeline (measured +10us).
                        den = den3.pop((h2, qb))
                        rec_f = rfp.tile([1, 512], F32, tag="recf")
                        nc.vector.reciprocal_approx_fast(rec_f[:], den[0:1, :])
                        rec_flat = nrm3.tile([1, 512], BF16, tag="rflat")
                        nc.vector.tensor_copy(rec_flat[:], rec_f[:])
                        rec3[(h2, qb)] = rec_flat

                    def norm3(h2, qb):
                        # PE row-broadcast of the reciprocal + DVE mul
                        bcp = psL.tile([64, 512], F32, tag="lg", name="bcp")
                        nc.tensor.matmul(
                            bcp[:], ones_sb[:], rec3.pop((h2, qb))[0:1, :],
                            start=True, stop=True,
                        )
                        nc.vector.tensor_mul(
                            vn_all[64 * h2:64 * (h2 + 1),
                                   S * j3 + 512 * qb:S * j3 + 512 * (qb + 1)],
                            va_tiles.pop(8 * j3 + 4 * h2 + qb)[:],
                            bcp[:],
                        )

                    def store(qb):
                        for tokb in range(4 * qb, 4 * qb + 4):
                            o_sb = obp.tile([128, HID], F32)
                            for ob in range(2):
                                opp = psP.tile([128, 512], F32, tag="pp")
                                for c in range(NPAIR):
                                    nc.tensor.matmul(
                                        opp[:],
                                        vn3[:, c, 128 * tokb:128 * (tokb + 1)],
                                        wo_sb[c][:, 512 * ob:512 * (ob + 1)],
                                        start=(c == 0), stop=(c == NPAIR - 1),
                                    )
                                nc.vector.tensor_copy(
                                    o_sb[:, 512 * ob:512 * (ob + 1)], opp[:])
                            nc.sync.dma_start(o[128 * tokb:128 * (tokb + 1), :], o_sb[:])

                    def pair3():
                        for qb in range(QB):
                            for h2 in range(2):
                                den = nrm3.tile([1, 512], F32, tag="dflat")
                                attn_block(j3, h2, qb, kt_sb3, qt_t[j3 % 2][h2],
                                           den[0:1, :])
                                den3[(h2, qb)] = den
                            if qb == 0:
                                proj_pair(*deferred_q3, tbs=[3])
                            else:
                                store(qb - 1)
                            chain3(0, qb)
                            chain3(1, qb)
                            norm3(0, qb)
                            norm3(1, qb)
                        store(QB - 1)

                    pair3()

        if n_iter > 1:
            with tc.For_i(0, n_iter, 1):
                body()
        else:
            body()

    nc.compile()
    return nc


def shard_inputs(x, w_qkv, w_o):
    x = np.asarray(x, dtype=np.float32)
    w_qkv = np.asarray(w_qkv, dtype=np.float32)
    w_o = np.asarray(w_o, dtype=np.float32)
    import ml_dtypes
    bf = ml_dtypes.bfloat16

    # w_qkv row (h*192 + c): c<64 q, 64<=c<128 k, 128<=c<192 v
    w3 = w_qkv.reshape(H, 3 * D, HID)
    wq_h = w3[:, 0:D, :]        # [H, D, HID]
    wk_h = w3[:, D:2 * D, :]
    wv_h = w3[:, 2 * D:3 * D, :]
    wo_t = w_o.T                # [HID(vals feat, h-major), HID(out)]

    cone8 = np.ones((128, 512), np.float32).astype(bf)
    cone = np.ones((1, 64), np.float32).astype(bf)
    in_maps = []
    for core in range(N_CORES):
        b, g = core // G, core % G
        hsel = slice(HG * g, HG * (g + 1))
        # all weights are pre-swizzled host-side into [128-partition, ...]
        # layouts so every input DMA is a plain 2D copy with >=1KB
        # contiguous lines (the previous per-128-col rearrange DMAs moved
        # 256B lines and dominated the kernel's startup latency)
        wq_g = wq_h[hsel].reshape(NPAIR, 2 * D, HID).transpose(0, 2, 1).reshape(NPAIR * HID, 128)
        wk_g = wk_h[hsel].reshape(NPAIR, 2 * D, HID).transpose(0, 2, 1).reshape(NPAIR * HID, 128)
        wq_p = wq_g.reshape(NPAIR, HID // 128, 128, 128).transpose(2, 0, 1, 3).reshape(128, NPAIR * HID)
        wk_p = wk_g.reshape(NPAIR, HID // 128, 128, 128).transpose(2, 0, 1, 3).reshape(128, NPAIR * HID)
        wv_g = wv_h[hsel].reshape(HG * D, HID).T        # [HID, 512]
        wv_p = wv_g.reshape(HT, 128, HG * D).transpose(1, 0, 2).reshape(128, HT * HG * D)
        wo_g = wo_t[HG * D * g:HG * D * (g + 1), :]     # [512, HID]
        wo_p = wo_g.reshape(NPAIR, 128, HID).transpose(1, 0, 2).reshape(128, NPAIR * HID)
        xb = np.ascontiguousarray(x[b].T).astype(bf)    # [HID, S]
        # x in token-block-major half strips: [128, (half, hid-tile, 1024)]
        xs = xb.reshape(HT, 128, 2, 1024).transpose(1, 2, 0, 3).reshape(128, HT * S)
        in_maps.append({
            "xt": np.ascontiguousarray(xs),
            "wq": np.ascontiguousarray(wq_p).astype(bf),
            "wk": np.ascontiguousarray(wk_p).astype(bf),
            "wv": np.ascontiguousarray(wv_p).astype(bf),
            "wo": np.ascontiguousarray(wo_p).astype(bf),
            "cone8": cone8, "cone": cone,
        })
    return in_maps


_NC_CACHE = {}


def get_nc(n_iter: int = 1):
    if n_iter not in _NC_CACHE:
        _NC_CACHE[n_iter] = build_nc(n_iter)
    return _NC_CACHE[n_iter]


def kernel(x, w_qkv, w_o):
    nc = get_nc(1)
    in_maps = shard_inputs(x, w_qkv, w_o)
    res = run_bass_kernel_spmd(nc, in_maps, list(range(N_CORES)))
    out = np.empty((B, S, HID), np.float32)
    for b in range(B):
        out[b] = res.results[G * b]["o"]
        for g in range(1, G):
            out[b] += res.results[G * b + g]["o"]
    return out



# revision 40
# speedup vs baseline: 1.0867x; 1.0060x over previous
"""Multi-head attention (B=4, S=2048, HID=1024, H=16, D=64) on 8 trn2 cores.

Sharding: batch x head-group (4 x 2). Core (2b+g) owns batch b and heads
8g..8g+7 over the FULL sequence: Q/K/V projections for its 8 heads,
attention, and a partial o-projection over its 512 value features. The host
sums the two partial o outputs per batch (the "all-reduce after o_proj"
done host-side) -- no duplicated projection work, no collectives.

Per-core dataflow (all matmuls full 128-partition moving operands, bf16,
fp32 PSUM accumulate -- avoids the half-bandwidth 64-partition moving path
and PE tiling-mode-switch drains):
  - K.T per pair packed [128=2x64 feat, token] bf16
  - Q.T per head zero-padded to [128, token] bf16 (other head's rows = 0),
    so logits contract over 128 partitions with the packed K stationary
  - V' in [token, (kt, head, 65)] bf16 with a ones column per head
    (softmax denominator falls out of the AV matmul as row 64)
  - logits L.T[k, q] in PSUM [128, 1024] (2 k-tiles); exp on ScalarE
  - AV accumulates vals'[65, 512] over 16 k-tiles; row 64 = denominator
  - denominator rows DMA'd from PSUM into a partition-major [8, 512] tile;
    ONE reciprocal per pair (free-size bound: 8x cheaper than reciprocal of
    broadcast tiles); bounced via DRAM back to a flat row, PE-broadcast,
    DVE multiply into vn
  - o_proj tail: vn (bf16) @ w_o.T shard (bf16) over 4 feature chunks

Schedule (this session's changes):
  - startup: ones constant loads first and feeds ~5us of PE clock-warmup
    junk; x loads in token-block strips so the pair-0 projection streams
    behind the DMA instead of waiting for the whole 4MB
  - pair 3 is pipelined per query-block: both heads' attention for qb,
    o-proj partial chunks 0-2 for qb's tokens, ACT-based reciprocal chains
    (1/x = exp(-ln x)) one qb late, then normalize + chunk-3 + add + store
    for qb-2; the output DMA streams during pair-3 attention
"""
import contextlib
import sys
sys.path.insert(0, "/opt/trn_rl_repo")
import numpy as np

import concourse.bass as bass
import concourse.mybir as mybir
import concourse.tile as tile
from concourse import bacc
from concourse.bass_utils import run_bass_kernel_spmd

F32 = mybir.dt.float32
F32R = mybir.dt.float32r
BF16 = mybir.dt.bfloat16
EXP = mybir.ActivationFunctionType.Exp

B, S, HID, H, D = 4, 2048, 1024, 16, 64
G = 2                  # head groups (cores per batch)
HG = H // G            # 8 heads per core
NPAIR = HG // 2        # 4 head pairs per core
HT = HID // 128        # 8 hid contraction tiles
TB = S // 512          # 4 proj token blocks
KT = S // 128          # 16 key-token tiles
QB = S // 512          # 4 query blocks of 512
N_CORES = 8


def build_nc(n_iter: int = 1):
    nc = bacc.Bacc(None, target_bir_lowering=False)

    # all inputs pre-swizzled host-side to [128, ...] partition-major so
    # every DMA is a contiguous-line 2D copy (see shard_inputs)
    xt = nc.dram_tensor("xt", [128, HT * S], BF16, kind="ExternalInput")
    wq = nc.dram_tensor("wq", [128, NPAIR * HID], BF16, kind="ExternalInput")
    wk = nc.dram_tensor("wk", [128, NPAIR * HID], BF16, kind="ExternalInput")
    wv = nc.dram_tensor("wv", [128, HT * HG * D], BF16, kind="ExternalInput")
    wo = nc.dram_tensor("wo", [128, NPAIR * HID], BF16, kind="ExternalInput")
    cone8 = nc.dram_tensor("cone8", [128, 512], BF16, kind="ExternalInput")
    cone = nc.dram_tensor("cone", [1, 64], BF16, kind="ExternalInput")
    o = nc.dram_tensor("o", [S, HID], F32, kind="ExternalOutput")

    with tile.TileContext(nc) as tc:
        def body():
            with contextlib.ExitStack() as _st:
                constp = _st.enter_context(tc.tile_pool(name="const", bufs=1))
                xtp = _st.enter_context(tc.tile_pool(name="xtp", bufs=1))
                vtp = _st.enter_context(tc.tile_pool(name="vtp", bufs=1))
                vnp = _st.enter_context(tc.tile_pool(name="vnp", bufs=1))
                wop = _st.enter_context(tc.tile_pool(name="wop", bufs=1))
                ktqp = _st.enter_context(tc.tile_pool(name="ktqp", bufs=1))

                # ones constant doubles as PE clock-warmup fodder: it is the
                # FIRST dma (128KB, lands ~1us) so the junk matmuls below can
                # warm the HAM clock gate while the 5.8MB of real inputs load
                ones8_sb = constp.tile([128, 512], BF16)
                nc.sync.dma_start(ones8_sb[:], cone8[:])
                ones_sb = constp.tile([1, 64], BF16)
                nc.sync.dma_start(ones_sb[:], cone[:])

                # x resident in SBUF, TOKEN-BLOCK-major: strip tb holds hid
                # tiles 0-7 for tokens [512*tb, 512*(tb+1)) contiguously, so
                # each strip's DMA write range is exact (no false deps) and
                # the pair-0 projection streams behind the x load strip by
                # strip instead of waiting for the whole 4MB.
                xt_all = xtp.tile([128, HT * S], BF16, name="xtall")

                def xt_at(ht, tok, width):
                    half, off = divmod(tok, 1024)
                    assert off + width <= 1024
                    base = HT * 1024 * half + 1024 * ht + off
                    return xt_all[:, base:base + width]

                wo_all = wop.tile([128, NPAIR * HID], BF16, name="woall")
                wo_sb = [wo_all[:, HID * c:HID * (c + 1)] for c in range(NPAIR)]

                # V' [token, (kt, head, 65)] bf16, resident in SBUF
                vt = vtp.tile([128, KT * HG * 65], BF16)
                vt4 = vt.rearrange("p (t h c) -> p t h c", h=HG, c=65)
                # normalized values [feat(128=2 heads), pair-chunk, token]
                vn_all = vnp.tile([128, NPAIR * S], BF16)

                # PE clock-warmup fodder: memset (no DMA dependency) so the
                # junk matmuls start at preamble-end (~6.5us) regardless of
                # how slow the chip-contended input DMA burst is
                warm_mm = constp.tile([128, 512], BF16, name="warmmm")
                nc.any.memset(warm_mm[:], 1.0)

                # persistent K/Q tiles, double-buffered across pairs.
                # qt_h zero-halves are memset once and never overwritten.
                kt_t = [ktqp.tile([128, S], BF16, name=f"kt{i}") for i in range(2)]
                qt_t = [[ktqp.tile([128, S], BF16, name=f"qt{i}{h2}") for h2 in range(2)]
                        for i in range(2)]
                for i in range(2):
                    nc.any.memset(qt_t[i][0][64:128, :], 0.0)
                    nc.any.memset(qt_t[i][1][0:64, :], 0.0)

                with contextlib.ExitStack() as _st2:
                    wvp = _st2.enter_context(tc.tile_pool(name="wvp", bufs=1))
                    wkp = _st2.enter_context(tc.tile_pool(name="wkp", bufs=2))
                    wqp = _st2.enter_context(tc.tile_pool(name="wqp", bufs=2))
                    ptp = _st2.enter_context(tc.tile_pool(name="ptp", bufs=3))
                    vap = _st2.enter_context(tc.tile_pool(name="vap", bufs=18))
                    nrm = _st2.enter_context(tc.tile_pool(name="nrm", bufs=2))
                    nrm3 = _st2.enter_context(tc.tile_pool(name="nrm3", bufs=4))
                    rfp = _st2.enter_context(tc.tile_pool(name="rfp", bufs=2))
                    obp = _st2.enter_context(tc.tile_pool(name="obp", bufs=3))
                    dramp = _st2.enter_context(tc.tile_pool(name="dramp", bufs=3, space="DRAM"))
                    psP = _st2.enter_context(tc.tile_pool(name="psP", bufs=2, space="PSUM"))
                    psL = _st2.enter_context(tc.tile_pool(name="psL", bufs=2, space="PSUM"))
                    psV = _st2.enter_context(tc.tile_pool(name="psV", bufs=2, space="PSUM"))
                    def proj_dma(j, w_dram, pool):
                        w_p = pool.tile([128, HID], BF16, tag="wp")
                        nc.sync.dma_start(w_p[:], w_dram[:, HID * j:HID * (j + 1)])
                        return w_p

                    # DMA queue order = emission order: pair-0 weights, then x
                    # in two token-half strips so the pair-0 projection (which
                    # consumes x token-block by token-block) starts after
                    # ~2.5MB instead of waiting for the whole 4MB x load.
                    wk0 = proj_dma(0, wk, wkp)
                    nc.sync.dma_start(xt_all[:, 0:HT * 1024], xt[:, 0:HT * 1024])
                    wq0 = proj_dma(0, wq, wqp)
                    nc.sync.dma_start(xt_all[:, HT * 1024:], xt[:, HT * 1024:])
                    wv_all = wvp.tile([128, HT * HG * D], BF16, name="wvall")
                    wv_sb = [wv_all[:, HG * D * t:HG * D * (t + 1)] for t in range(HT)]
                    nc.sync.dma_start(wv_all[:], wv[:])
                    nc.sync.dma_start(wo_all[:], wo[:])

                    # prewarm the activation tables while input DMAs are in
                    # flight -- Ln first pins the natural_log_exp_and_others
                    # set, which also serves every Exp (no mid-kernel reload)
                    warm = nrm.tile([1, 8], BF16, tag="warm")
                    nc.scalar.activation(warm[:], ones8_sb[0:1, 0:8],
                                         mybir.ActivationFunctionType.Ln)
                    nc.scalar.activation(warm[:], ones8_sb[0:1, 0:8], EXP, scale=0.0)
                    # warm the PE HAM clock gate on the ones constant (lands
                    # ~1us, far ahead of x): junk matmuls sized to bridge the
                    # chip-HBM-bound input load (~12us until the first x half
                    # lands) so the first projection runs at 2.4GHz
                    for _ in range(36):
                        jps = psP.tile([128, 512], F32, tag="pp")
                        nc.tensor.matmul(
                            jps[:], warm_mm[:, 0:128], warm_mm[:],
                            start=True, stop=True,
                        )

                    def v_proj(tokt):
                        vps = psP.tile([128, 512], F32, tag="pp")
                        for ht in range(HT):
                            nc.tensor.matmul(
                                vps[:],
                                xt_at(ht, 128 * tokt, 128),
                                wv_sb[ht][:],
                                start=(ht == 0), stop=(ht == HT - 1),
                            )
                        nc.vector.tensor_copy(
                            vt4[:, tokt, :, 0:64],
                            vps.rearrange("p (h c) -> p h c", c=64),
                        )
                        nc.vector.tensor_copy(vt4[:, tokt, :, 64], ones8_sb[:, 0:8])

                    # ---- pair pipeline: K/Q proj + attention + normalize ----
                    va_tiles = {}

                    def proj_pair(w_p, evac, tbs=range(TB)):
                        for tb in tbs:
                            pps = psP.tile([128, 512], F32, tag="pp")
                            for ht in range(HT):
                                nc.tensor.matmul(
                                    pps[:],
                                    w_p[:, 128 * ht:128 * (ht + 1)],
                                    xt_at(ht, 512 * tb, 512),
                                    start=(ht == 0), stop=(ht == HT - 1),
                                )
                            evac(tb, pps)

                    def attn_block(j, h2, qb, kt_sb, qt_sb, den_out, v_inter=False):
                        h = 2 * j + h2
                        first_mm = None
                        vals = psV.tile([65, 512], F32, tag="vv")
                        for ktp2 in range(KT // 2):
                            lg = psL.tile([128, 1024], F32, tag="lg")
                            for u in range(2):
                                kt = 2 * ktp2 + u
                                mm = nc.tensor.matmul(
                                    lg[:, 512 * u:512 * (u + 1)],
                                    kt_sb[:, 128 * kt:128 * (kt + 1)],
                                    qt_sb[:, 512 * qb:512 * (qb + 1)],
                                    start=True, stop=True,
                                )
                                if first_mm is None:
                                    first_mm = mm
                            pt = ptp.tile([128, 1024], BF16, tag="pt")
                            nc.scalar.activation(pt[:], lg[:], EXP, scale=0.125)
                            if v_inter:
                                # first attention block: V' projection for these
                                # two k-tiles lands just ahead of their AV use
                                v_proj(2 * ktp2)
                                v_proj(2 * ktp2 + 1)
                            for u in range(2):
                                kt = 2 * ktp2 + u
                                nc.tensor.matmul(
                                    vals[:],
                                    vt4[:, kt, h, :],
                                    pt[:, 512 * u:512 * (u + 1)],
                                    start=(ktp2 == 0 and u == 0),
                                    stop=(ktp2 == KT // 2 - 1 and u == 1),
                                )
                        nc.vector.tensor_copy(den_out, vals[64:65, :])
                        va = vap.tile([64, 512], BF16, tag="va")
                        nc.vector.tensor_copy(va[:], vals[0:64, :])
                        va_tiles[8 * j + 4 * h2 + qb] = va
                        return first_mm

                    rec_drams = {}
                    rec_flats = {}

                    def chain(j, h2, den_flat, use_act=False):
                        # reciprocal of this half-pair's 4 denominator rows.
                        # engine ops cannot address partitions 1..31, so either
                        # bounce through DRAM to partition-major for the DVE
                        # reciprocal, or (for the last chain, when the scalar
                        # engine has gone idle) reciprocal the flat row on ACT.
                        if use_act:
                            # 1/x = exp(-ln x): two ACT ops on the flat row --
                            # the scalar engine is idle by the last chain and
                            # this skips two DMA bounce hops on the o-proj
                            # critical path (both fns live in the preloaded
                            # natural_log_exp table set)
                            lnt = nrm.tile([1, 4 * 512], F32, tag="lnt")
                            nc.scalar.activation(lnt[:], den_flat[:],
                                                 mybir.ActivationFunctionType.Ln)
                            rec_flat = nrm.tile([1, 4 * 512], BF16, tag="rflat")
                            nc.scalar.activation(rec_flat[:], lnt[:], EXP, scale=-1.0)
                            rec_flats[(j, h2)] = rec_flat
                        else:
                            rec_dram = dramp.tile([4, 512], BF16, tag="rdram")
                            den_dram = dramp.tile([4, 512], BF16, tag="ddram")
                            nc.sync.dma_start(
                                den_dram.rearrange("r c -> (r c)")[None, :], den_flat[0:1, :]
                            )
                            den_sq = nrm.tile([4, 512], BF16, tag="dsq")
                            nc.sync.dma_start(den_sq[:], den_dram[:])
                            rec_sq = nrm.tile([4, 512], BF16, tag="rsq")
                            with nc.allow_low_precision(reason="denominator reciprocal in bf16"):
                                nc.vector.reciprocal(rec_sq[:], den_sq[:])
                            nc.sync.dma_start(rec_dram[:], rec_sq[:])
                            rec_drams[(j, h2)] = rec_dram

                    def normalize(j, h2):
                        # runs a half-pair or more late: the reciprocal chain
                        # has had a full attention half to complete, so nothing
                        # here blocks the in-order engine streams. The last
                        # chain instead broadcasts via a PE matmul straight
                        # from the flat reciprocal row (PE is idle by then and
                        # this skips two DMA hops on the o-proj critical path).
                        rec_flat = rec_flats.pop((j, h2), None)
                        rec_dram = None if rec_flat is not None else rec_drams.pop((j, h2))
                        for qb in range(QB):
                            if rec_flat is not None:
                                bcp = psL.tile([64, 512], F32, tag="lg", name="bcp")
                                nc.tensor.matmul(
                                    bcp[:], ones_sb[:],
                                    rec_flat[0:1, 512 * qb:512 * (qb + 1)],
                                    start=True, stop=True,
                                )
                                bop = bcp
                            else:
                                bcs = nrm.tile([64, 512], BF16, tag="bcs")
                                nc.sync.dma_start(
                                    bcs[:], rec_dram[qb:qb + 1, :].broadcast_to([64, 512])
                                )
                                bop = bcs
                            nc.vector.tensor_mul(
                                vn_all[64 * h2:64 * (h2 + 1),
                                       S * j + 512 * qb:S * j + 512 * (qb + 1)],
                                va_tiles.pop(8 * j + 4 * h2 + qb)[:],
                                bop[:],
                            )

                    def k_evac_f(kt_sb):
                        def k_evac(tb, pps):
                            nc.vector.tensor_copy(kt_sb[:, 512 * tb:512 * (tb + 1)], pps[:])
                        return k_evac

                    def q_evac_f(jj):
                        def q_evac(tb, pps):
                            nc.vector.tensor_copy(
                                qt_t[jj % 2][0][0:64, 512 * tb:512 * (tb + 1)], pps[0:64, :])
                            nc.vector.tensor_copy(
                                qt_t[jj % 2][1][64:128, 512 * tb:512 * (tb + 1)], pps[64:128, :])
                        return q_evac

                    proj_pair(wk0, k_evac_f(kt_t[0]))
                    proj_pair(wq0, q_evac_f(0))
                    for j in range(NPAIR - 1):
                        kt_sb = kt_t[j % 2]
                        for h2 in range(2):
                            den_flat = nrm.tile([1, 4 * 512], BF16, tag="dflat")
                            for qb in range(QB):
                                attn_block(j, h2, qb, kt_sb, qt_t[j % 2][h2],
                                           den_flat[0:1, 512 * qb:512 * (qb + 1)],
                                           v_inter=(j == 0 and h2 == 0 and qb == 0))
                            chain(j, h2, den_flat, use_act=False)
                        normalize(j, 0)
                        proj_pair(proj_dma(j + 1, wk, wkp), k_evac_f(kt_t[(j + 1) % 2]))
                        wq_n = proj_dma(j + 1, wq, wqp)
                        if j < NPAIR - 2:
                            proj_pair(wq_n, q_evac_f(j + 1))
                        else:
                            # defer pair-3's Q token-block 3 into pair-3 qb0's
                            # window: that window is exp-paced with no store to
                            # fill PE slack (it's only needed by qb3's blocks)
                            proj_pair(wq_n, q_evac_f(j + 1), tbs=[0, 1, 2])
                            deferred_q3 = (wq_n, q_evac_f(j + 1))
                        normalize(j, 1)

                    # ---- pair 3, pipelined per query-block ----
                    # No projections remain to fill PE slack, so the o-proj
                    # runs FUSED per token-block here (all 4 pair chunks in
                    # one PSUM accumulation, one f32 evacuation, one store) as
                    # soon as this qb's pair-3 values are normalized. 4
                    # matmuls per unit vs one DVE copy keeps the store stream
                    # PE-paced; each qb's store overlaps the next qb's
                    # attention, leaving only the last group's ~8us serial.
                    j3 = NPAIR - 1
                    kt_sb3 = kt_t[j3 % 2]
                    vn3 = vn_all.rearrange("p (c s) -> p c s", c=NPAIR)
                    rec3 = {}
                    den3 = {}

                    def chain3(h2, qb):
                        # single-op DVE approximate reciprocal (~18 bits, no
                        # DRAM bounce). NOT Ln+Exp on ACT: Ln lives in a
                        # different activation-table set than the exp stream's,
                        # so a mid-kernel Ln forces two 1.3us ACT_TABLE_LOADs
                        # and stalls the exp pipeline (measured +10us).
                        den = den3.pop((h2, qb))
                        rec_f = rfp.tile([1, 512], F32, tag="recf")
                        nc.vector.reciprocal_approx_fast(rec_f[:], den[0:1, :])
                        rec_flat = nrm3.tile([1, 512], BF16, tag="rflat")
                        nc.vector.tensor_copy(rec_flat[:], rec_f[:])
                        rec3[(h2, qb)] = rec_flat

                    def norm3(h2, qb):
                        # PE row-broadcast of the reciprocal + DVE mul
                        bcp = psL.tile([64, 512], F32, tag="lg", name="bcp")
                        nc.tensor.matmul(
                            bcp[:], ones_sb[:], rec3.pop((h2, qb))[0:1, :],
                            start=True, stop=True,
                        )
                        nc.vector.tensor_mul(
                            vn_all[64 * h2:64 * (h2 + 1),
                                   S * j3 + 512 * qb:S * j3 + 512 * (qb + 1)],
                            va_tiles.pop(8 * j3 + 4 * h2 + qb)[:],
                            bcp[:],
                        )

                    def store(qb):
                        for tokb in range(4 * qb, 4 * qb + 4):
                            o_sb = obp.tile([128, HID], F32)
                            for ob in range(2):
                                opp = psP.tile([128, 512], F32, tag="pp")
                                for c in range(NPAIR):
                                    nc.tensor.matmul(
                                        opp[:],
                                        vn3[:, c, 128 * tokb:128 * (tokb + 1)],
                                        wo_sb[c][:, 512 * ob:512 * (ob + 1)],
                                        start=(c == 0), stop=(c == NPAIR - 1),
                                    )
                                nc.vector.tensor_copy(
                                    o_sb[:, 512 * ob:512 * (ob + 1)], opp[:])
                            nc.sync.dma_start(o[128 * tokb:128 * (tokb + 1), :], o_sb[:])

                    def pair3():
                        # h2=0's reciprocal chain runs on DVE during h2=1's
                        # attention, so norm3(0) never stalls the PE; the
                        # store (or the deferred Q3 chunk for qb0) covers
                        # h2=1's chain latency.
                        for qb in range(QB):
                            for h2 in range(2):
                                den = nrm3.tile([1, 512], F32, tag="dflat")
                                attn_block(j3, h2, qb, kt_sb3, qt_t[j3 % 2][h2],
                                           den[0:1, :])
                                den3[(h2, qb)] = den
                                chain3(h2, qb)
                            norm3(0, qb)
                            if qb == 0:
                                proj_pair(*deferred_q3, tbs=[3])
                            else:
                                store(qb - 1)
                            norm3(1, qb)
                        store(QB - 1)

                    pair3()

        if n_iter > 1:
            with tc.For_i(0, n_iter, 1):
                body()
        else:
            body()

    nc.compile()
    return nc


def shard_inputs(x, w_qkv, w_o):
    x = np.asarray(x, dtype=np.float32)
    w_qkv = np.asarray(w_qkv, dtype=np.float32)
    w_o = np.asarray(w_o, dtype=np.float32)
    import ml_dtypes
    bf = ml_dtypes.bfloat16

    # w_qkv row (h*192 + c): c<64 q, 64<=c<128 k, 128<=c<192 v
    w3 = w_qkv.reshape(H, 3 * D, HID)
    wq_h = w3[:, 0:D, :]        # [H, D, HID]
    wk_h = w3[:, D:2 * D, :]
    wv_h = w3[:, 2 * D:3 * D, :]
    wo_t = w_o.T                # [HID(vals feat, h-major), HID(out)]

    cone8 = np.ones((128, 512), np.float32).astype(bf)
    cone = np.ones((1, 64), np.float32).astype(bf)
    in_maps = []
    for core in range(N_CORES):
        b, g = core // G, core % G
        hsel = slice(HG * g, HG * (g + 1))
        # all weights are pre-swizzled host-side into [128-partition, ...]
        # layouts so every input DMA is a plain 2D copy with >=1KB
        # contiguous lines (the previous per-128-col rearrange DMAs moved
        # 256B lines and dominated the kernel's startup latency)
        wq_g = wq_h[hsel].reshape(NPAIR, 2 * D, HID).transpose(0, 2, 1).reshape(NPAIR * HID, 128)
        wk_g = wk_h[hsel].reshape(NPAIR, 2 * D, HID).transpose(0, 2, 1).reshape(NPAIR * HID, 128)
        wq_p = wq_g.reshape(NPAIR, HID // 128, 128, 128).transpose(2, 0, 1, 3).reshape(128, NPAIR * HID)
        wk_p = wk_g.reshape(NPAIR, HID // 128, 128, 128).transpose(2, 0, 1, 3).reshape(128, NPAIR * HID)
        wv_g = wv_h[hsel].reshape(HG * D, HID).T        # [HID, 512]
        wv_p = wv_g.reshape(HT, 128, HG * D).transpose(1, 0, 2).reshape(128, HT * HG * D)
        wo_g = wo_t[HG * D * g:HG * D * (g + 1), :]     # [512, HID]
        wo_p = wo_g.reshape(NPAIR, 128, HID).transpose(1, 0, 2).reshape(128, NPAIR * HID)
        xb = np.ascontiguousarray(x[b].T).astype(bf)    # [HID, S]
        # x in token-block-major half strips: [128, (half, hid-tile, 1024)]
        xs = xb.reshape(HT, 128, 2, 1024).transpose(1, 2, 0, 3).reshape(128, HT * S)
        in_maps.append({
            "xt": np.ascontiguousarray(xs),
            "wq": np.ascontiguousarray(wq_p).astype(bf),
            "wk": np.ascontiguousarray(wk_p).astype(bf),
            "wv": np.ascontiguousarray(wv_p).astype(bf),
            "wo": np.ascontiguousarray(wo_p).astype(bf),
            "cone8": cone8, "cone": cone,
        })
    return in_maps


_NC_CACHE = {}


def get_nc(n_iter: int = 1):
    if n_iter not in _NC_CACHE:
        _NC_CACHE[n_iter] = build_nc(n_iter)
    return _NC_CACHE[n_iter]


def kernel(x, w_qkv, w_o):
    nc = get_nc(1)
    in_maps = shard_inputs(x, w_qkv, w_o)
    res = run_bass_kernel_spmd(nc, in_maps, list(range(N_CORES)))
    out = np.empty((B, S, HID), np.float32)
    for b in range(B):
        out[b] = res.results[G * b]["o"]
        for g in range(1, G):
            out[b] += res.results[G * b + g]["o"]
    return out

